# revision 1
# baseline (speedup 1.0000x reference)
"""HGRN BitAttention Trainium2 kernel (8-core SPMD, token-sharded).

Sharding: core c handles batch c//2, sequence half c%2 (1024 tokens).
The HGRN recurrence carry h[t=1023] crosses the half boundary via a tiny
pair-AllReduce; masks make the program uniform (SPMD).

BitLinear trick: activations are quantized to integers in [-127,127] and
weights to {-1,0,1} — both exact in bf16 — so all four projections run as
exact-integer bf16 matmuls with fp32 PSUM accumulation; the (1/s_token)(1/ws)
scales are applied to the fp32 outputs.

Layouts: i/f projections produce feature-major [d_out, tok] tiles so the
recurrence runs along the free axis via tensor_tensor_scan; g and the output
projection run token-major. h is transposed back per 128-token tile on the PE.
"""

import numpy as np
import ml_dtypes

import concourse.bass as bass
import concourse.bacc as bacc
import concourse.mybir as mybir
import concourse.tile as tile
from concourse.bass_utils import run_bass_kernel_spmd

F32 = mybir.dt.float32
BF16 = mybir.dt.bfloat16
I32 = mybir.dt.int32
AF = mybir.ActivationFunctionType
OP = mybir.AluOpType

B, L, D = 4, 2048, 2048
NCORES = 8
TPC = L // 2          # tokens per core = 1024
NTT = TPC // 128      # 8 token tiles per core
KT = D // 128         # 16 k tiles
MT = D // 128         # 16 m tiles
MBLK = 8              # m-blocks of 256 for i/f weights
NB = 4                # 512-wide n chunks (token-major matmuls)
NCH = 4               # tail token-chunks of 256
EPS = 1e-5


def build_nc():
    nc = bacc.Bacc("TRN2", target_bir_lowering=False, debug=False,
                   num_devices=NCORES)

    x_d = nc.dram_tensor("x", [TPC, D], F32, kind="ExternalInput")
    wit_d = nc.dram_tensor("wit", [MBLK, 128, KT, 256], BF16, kind="ExternalInput")
    wft_d = nc.dram_tensor("wft", [MBLK, 128, KT, 256], BF16, kind="ExternalInput")
    wgt_d = nc.dram_tensor("wgt", [D, D], BF16, kind="ExternalInput")
    wot_d = nc.dram_tensor("wot", [D, D], BF16, kind="ExternalInput")
    gw_d = nc.dram_tensor("gw", [1, D], F32, kind="ExternalInput")
    id_d = nc.dram_tensor("id128", [128, 128], F32, kind="ExternalInput")
    me_d = nc.dram_tensor("mask_even", [128, 1], F32, kind="ExternalInput")
    mo_d = nc.dram_tensor("mask_odd", [128, 1], F32, kind="ExternalInput")
    rws_d = nc.dram_tensor("rws", [128, 5], F32, kind="ExternalInput")
    out_d = nc.dram_tensor("out", [TPC, D], F32, kind="ExternalOutput")

    with tile.TileContext(nc) as tc:
        with (
            tc.tile_pool(name="const", bufs=1) as cp,
            tc.tile_pool(name="xq", bufs=1) as xqp,
            tc.tile_pool(name="hp", bufs=1) as hp,
            tc.tile_pool(name="dram", bufs=1, space="DRAM") as dram,
        ):
            # ---- constants ----
            idt = cp.tile([128, 128], F32)
            nc.sync.dma_start(idt[:], id_d.ap())
            me = cp.tile([128, 1], F32)
            nc.sync.dma_start(me[:], me_d.ap())
            mo = cp.tile([128, 1], F32)
            nc.sync.dma_start(mo[:], mo_d.ap())
            rws = cp.tile([128, 5], F32)
            nc.sync.dma_start(rws[:], rws_d.ap())
            rwsi, rwsf, rwsfn, rwsg, rwso = (rws[:, i:i + 1] for i in range(5))
            epsb = cp.tile([128, 1], F32)
            nc.vector.memset(epsb[:], EPS)
            zeros = cp.tile([128, TPC], F32)
            nc.vector.memset(zeros[:], 0.0)
            ones1 = cp.tile([1, 128], F32)
            nc.vector.memset(ones1[:], 1.0)

            srec = cp.tile([128, NTT], F32)     # (1/s_x) per token tile col
            sgcol = cp.tile([128, NTT], F32)    # (1/s_x)*(1/ws_g)
            coall = cp.tile([128, NTT], F32)    # (1/s_o)*(1/ws_o)
            bnd = cp.tile([128, MT], F32)
            bnd2 = cp.tile([128, MT], F32)
            carried = cp.tile([128, MT], F32)
            S = cp.tile([128, TPC], F32)        # (1/s_x) broadcast, feature-major
            gwb = cp.tile([128, D], F32)        # g_norm_weight broadcast

            xqT = xqp.tile([128, KT * TPC], BF16)  # [d_in-major] quantized x
            xqT3 = xqT[:].rearrange("p (k t) -> p k t", k=KT)
            hs = [None] * MT
            fcs = [None] * MT

            # ================= Phase X: normalize + quantize x =================
            with (
                tc.tile_pool(name="xin", bufs=2) as xin,
                tc.tile_pool(name="xw", bufs=2) as xw,
                tc.tile_pool(name="psx", bufs=1, space="PSUM") as psx,
            ):
                for tt in range(NTT):
                    xt = xin.tile([128, D], F32)
                    nc.sync.dma_start(xt[:], x_d.ap()[tt * 128:(tt + 1) * 128, :])
                    scr = xw.tile([128, D], F32)
                    ssum = xw.tile([128, 1], F32)
                    nc.scalar.activation(scr[:], xt[:], AF.Square, accum_out=ssum[:])
                    std = xw.tile([128, 1], F32)
                    nc.scalar.activation(std[:], ssum[:], AF.Sqrt,
                                         bias=epsb[:], scale=1.0 / D)
                    rstd = xw.tile([128, 1], F32)
                    nc.vector.reciprocal(rstd[:], std[:])
                    xn = xw.tile([128, D], F32)
                    nc.scalar.mul(xn[:], xt[:], rstd[:])
                    mxn = xw.tile([128, 1], F32)
                    nc.vector.tensor_reduce(mxn[:], xn[:], mybir.AxisListType.X,
                                            OP.max, apply_absolute_value=True)
                    nc.vector.tensor_scalar_max(mxn[:], mxn[:], EPS)
                    nc.vector.tensor_scalar_mul(srec[:, tt:tt + 1], mxn[:], 1.0 / 127.0)
                    sst = xw.tile([128, 1], F32)
                    nc.vector.reciprocal(sst[:], mxn[:])
                    nc.vector.tensor_scalar_mul(sst[:], sst[:], 127.0)
                    qi = xw.tile([128, D], I32)
                    nc.scalar.activation(qi[:], xn[:], AF.Identity, scale=sst[:])
                    qb = xw.tile([128, D], BF16)
                    nc.vector.tensor_copy(qb[:], qi[:])
                    nc.sync.dma_start_transpose(
                        xqT3[:, :, tt * 128:(tt + 1) * 128], qb[:])

                # combined g scale per token tile: (1/s)*(1/ws_g)
                nc.vector.tensor_scalar_mul(sgcol[:], srec[:], rwsg)

                # S = broadcast of (1/s) to [128, TPC] feature-major
                srd = dram.tile([1, TPC], F32)
                nc.sync.dma_start(
                    srd[:].rearrange("o (t p) -> (o p) t", p=128), srec[:])
                srow = cp.tile([1, TPC], F32)
                nc.sync.dma_start(srow[:], srd[:])
                for c in range(2):
                    pS = psx.tile([128, 512], F32)
                    nc.tensor.matmul(pS[:], ones1[:], srow[:, c * 512:(c + 1) * 512],
                                     start=True, stop=True)
                    nc.scalar.copy(S[:, c * 512:(c + 1) * 512], pS[:])

                # gw broadcast to [128, D]
                gwrow = cp.tile([1, D], F32)
                nc.sync.dma_start(gwrow[:], gw_d.ap())
                for c in range(NB):
                    pG = psx.tile([128, 512], F32)
                    nc.tensor.matmul(pG[:], ones1[:], gwrow[:, c * 512:(c + 1) * 512],
                                     start=True, stop=True)
                    nc.scalar.copy(gwb[:, c * 512:(c + 1) * 512], pG[:])

            # ============ Phase P: i/f projections + scans (feature-major) ============
            fcp_ctx = tc.tile_pool(name="fcp", bufs=1)
            fcp = fcp_ctx.__enter__()
            with (
                tc.tile_pool(name="wfi", bufs=1) as wfi,
                tc.tile_pool(name="pw", bufs=1) as pw,
                tc.tile_pool(name="psp", bufs=2, space="PSUM") as psp,
            ):
                for mb in range(MBLK):
                    wi_sb = wfi.tile([128, KT * 256], BF16)
                    nc.sync.dma_start(
                        wi_sb[:], wit_d.ap()[mb].rearrange("p k c -> p (k c)"))
                    wf_sb = wfi.tile([128, KT * 256], BF16)
                    nc.sync.dma_start(
                        wf_sb[:], wft_d.ap()[mb].rearrange("p k c -> p (k c)"))
                    for j in range(2):
                        m = mb * 2 + j
                        psi0 = psp.tile([128, 512], F32)
                        psi1 = psp.tile([128, 512], F32)
                        psf0 = psp.tile([128, 512], F32)
                        psf1 = psp.tile([128, 512], F32)
                        for k in range(KT):
                            li = wi_sb[:, k * 256 + j * 128: k * 256 + j * 128 + 128]
                            lf = wf_sb[:, k * 256 + j * 128: k * 256 + j * 128 + 128]
                            st, sp = (k == 0), (k == KT - 1)
                            nc.tensor.matmul(psi0[:], li, xqT[:, k * TPC: k * TPC + 512],
                                             start=st, stop=sp)
                            nc.tensor.matmul(psi1[:], li, xqT[:, k * TPC + 512: (k + 1) * TPC],
                                             start=st, stop=sp)
                            nc.tensor.matmul(psf0[:], lf, xqT[:, k * TPC: k * TPC + 512],
                                             start=st, stop=sp)
                            nc.tensor.matmul(psf1[:], lf, xqT[:, k * TPC + 512: (k + 1) * TPC],
                                             start=st, stop=sp)
                        tmpf = pw.tile([128, TPC], F32)
                        nc.vector.tensor_tensor(tmpf[:, 0:512], psf0[:], S[:, 0:512], OP.mult)
                        nc.vector.tensor_tensor(tmpf[:, 512:TPC], psf1[:], S[:, 512:TPC], OP.mult)
                        G = pw.tile([128, TPC], F32)
                        nc.scalar.activation(G[:], tmpf[:], AF.Sigmoid, scale=rwsfn)
                        F = pw.tile([128, TPC], F32)
                        nc.scalar.activation(F[:], tmpf[:], AF.Sigmoid, scale=rwsf)
                        hs[m] = hp.tile([128, TPC], F32, name=f"h_{m}")
                        fcs[m] = fcp.tile([128, TPC], mybir.dt.float16, name=f"fc_{m}")
                        tmpi = pw.tile([128, TPC], F32, name="tmpf")
                        nc.vector.tensor_tensor(tmpi[:, 0:512], psi0[:], S[:, 0:512], OP.mult)
                        nc.vector.tensor_tensor(tmpi[:, 512:TPC], psi1[:], S[:, 512:TPC], OP.mult)
                        sgi = pw.tile([128, TPC], F32, name="SiL")
                        nc.scalar.activation(sgi[:], tmpi[:], AF.Sigmoid, scale=rwsi)
                        SiL2 = pw.tile([128, TPC], F32, name="SiL2")
                        nc.vector.scalar_tensor_tensor(SiL2[:], tmpi[:], rwsi,
                                                       sgi[:], OP.mult, OP.mult)
                        Iin = pw.tile([128, TPC], F32, name="Iin")
                        nc.vector.tensor_tensor(Iin[:], SiL2[:], G[:], OP.mult)
                        nc.vector.tensor_tensor_scan(hs[m][:], F[:], Iin[:], 0.0,
                                                     OP.mult, OP.add)
                        nc.vector.tensor_tensor_scan(fcs[m][:], F[:], zeros[:], 1.0,
                                                     OP.mult, OP.add)
                        nc.vector.tensor_copy(bnd[:, m:m + 1], hs[m][:, TPC - 1:TPC])

            # ================= Phase C: carry exchange + fixup =================
            nc.vector.tensor_scalar_mul(bnd2[:], bnd[:], me[:])
            cin = dram.tile([128, MT], F32)
            cout = dram.tile([128, MT], F32)
            nc.sync.dma_start(cin[:], bnd2[:])
            nc.gpsimd.collective_compute(
                "AllReduce", OP.add,
                replica_groups=[[0, 1], [2, 3], [4, 5], [6, 7]],
                ins=[cin.opt()], outs=[cout.opt()],
            )
            carry_sb = cp.tile([128, MT], F32)
            nc.sync.dma_start(carry_sb[:], cout[:])
            nc.vector.tensor_scalar_mul(carried[:], carry_sb[:], mo[:])
            for m in range(MT):
                nc.vector.scalar_tensor_tensor(
                    hs[m][:], fcs[m][:], carried[:, m:m + 1], hs[m][:],
                    OP.mult, OP.add)

            fcp_ctx.__exit__(None, None, None)

            # ================= Phase T: gate + output projection =================
            with (
                tc.tile_pool(name="wst", bufs=2) as wst,
                tc.tile_pool(name="tw", bufs=1) as tw,
                tc.tile_pool(name="oqt", bufs=1) as oqtp,
                tc.tile_pool(name="psg", bufs=1, space="PSUM") as psgp,
            ):
                for ch in range(NCH):
                    psg = [[psgp.tile([128, 512], F32, name=f"psg_{t2}_{nb}")
                            for nb in range(NB)] for t2 in range(2)]
                    # g projection, token-major
                    for k in range(KT):
                        wg_k = wst.tile([128, D], BF16, name="wg_k")
                        nc.sync.dma_start(wg_k[:], wgt_d.ap()[k * 128:(k + 1) * 128, :])
                        st, sp = (k == 0), (k == KT - 1)
                        for t2 in range(2):
                            tti = ch * 2 + t2
                            lhsT = xqT3[:, k, tti * 128:(tti + 1) * 128]
                            for nb in range(NB):
                                nc.tensor.matmul(psg[t2][nb][:], lhsT,
                                                 wg_k[:, nb * 512:(nb + 1) * 512],
                                                 start=st, stop=sp)
                    oqT = oqtp.tile([128, KT * 256], BF16)
                    oqT3 = oqT[:].rearrange("p (k t) -> p k t", k=KT)
                    for t2 in range(2):
                        tti = ch * 2 + t2
                        gsc = tw.tile([128, D], F32)
                        for nb in range(NB):
                            nc.scalar.mul(gsc[:, nb * 512:(nb + 1) * 512],
                                          psg[t2][nb][:], sgcol[:, tti:tti + 1])
                        scr2 = tw.tile([128, D], F32)
                        ssg = tw.tile([128, 1], F32)
                        nc.vector.scalar_tensor_tensor(scr2[:], gsc[:], 1.0, gsc[:],
                                                       OP.mult, OP.mult,
                                                       accum_out=ssg[:])
                        stdg = tw.tile([128, 1], F32)
                        nc.scalar.activation(stdg[:], ssg[:], AF.Sqrt,
                                             bias=epsb[:], scale=1.0 / D)
                        rg = tw.tile([128, 1], F32)
                        nc.vector.reciprocal(rg[:], stdg[:])
                        # h transpose + h*sigmoid(h), token-major
                        hsig = tw.tile([128, D], F32)
                        for grp in range(4):
                            pt = psgp.tile([128, 512], F32,
                                           name=f"psg_{t2}_{grp}")
                            for m4 in range(4):
                                m = grp * 4 + m4
                                nc.tensor.matmul(
                                    pt[:, m4 * 128:(m4 + 1) * 128],
                                    hs[m][:, tti * 128:(tti + 1) * 128], idt[:],
                                    is_transpose=True, start=True, stop=True)
                            sgb = tw.tile([128, 512], F32, name="sgb")
                            nc.scalar.activation(sgb[:], pt[:], AF.Sigmoid)
                            nc.vector.tensor_tensor(
                                hsig[:, grp * 512:(grp + 1) * 512], pt[:], sgb[:],
                                OP.mult)
                        # o = (gsc * gwb) * rg * hsig
                        nc.vector.tensor_tensor(gsc[:], gsc[:], gwb[:], OP.mult)
                        o = tw.tile([128, D], F32)
                        nc.vector.scalar_tensor_tensor(o[:], gsc[:], rg[:], hsig[:],
                                                       OP.mult, OP.mult)
                        # quantize o (two-step, matching reference association)
                        sso = tw.tile([128, 1], F32)
                        nc.vector.scalar_tensor_tensor(scr2[:], o[:], 1.0, o[:],
                                                       OP.mult, OP.mult,
                                                       accum_out=sso[:])
                        stdo = tw.tile([128, 1], F32)
                        nc.scalar.activation(stdo[:], sso[:], AF.Sqrt,
                                             bias=epsb[:], scale=1.0 / D)
                        rstdo = tw.tile([128, 1], F32)
                        nc.vector.reciprocal(rstdo[:], stdo[:])
                        on = tw.tile([128, D], F32, name="gsc")
                        nc.scalar.mul(on[:], o[:], rstdo[:])
                        mxno = tw.tile([128, 1], F32)
                        nc.vector.tensor_reduce(mxno[:], on[:], mybir.AxisListType.X,
                                                OP.max, apply_absolute_value=True)
                        nc.vector.tensor_scalar_max(mxno[:], mxno[:], EPS)
                        cot = tw.tile([128, 1], F32)
                        nc.vector.tensor_scalar_mul(cot[:], mxno[:], 1.0 / 127.0)
                        nc.vector.tensor_scalar_mul(coall[:, tti:tti + 1], cot[:], rwso)
                        ssto = tw.tile([128, 1], F32)
                        nc.vector.reciprocal(ssto[:], mxno[:])
                        nc.vector.tensor_scalar_mul(ssto[:], ssto[:], 127.0)
                        oqi = tw.tile([128, D], I32)
                        nc.vector.tensor_scalar_mul(oqi[:], on[:], ssto[:])
                        oqb = tw.tile([128, D], BF16)
                        nc.vector.tensor_copy(oqb[:], oqi[:])
                        nc.sync.dma_start_transpose(
                            oqT3[:, :, t2 * 128:(t2 + 1) * 128], oqb[:])
                    # output projection, token-major (fresh tiles, same banks)
                    pso = [[psgp.tile([128, 512], F32, name=f"psg_{t2}_{nb}")
                            for nb in range(NB)] for t2 in range(2)]
                    for k in range(KT):
                        wo_k = wst.tile([128, D], BF16, name="wo_k")
                        nc.sync.dma_start(wo_k[:], wot_d.ap()[k * 128:(k + 1) * 128, :])
                        st, sp = (k == 0), (k == KT - 1)
                        for t2 in range(2):
                            lhsT = oqT3[:, k, t2 * 128:(t2 + 1) * 128]
                            for nb in range(NB):
                                nc.tensor.matmul(pso[t2][nb][:], lhsT,
                                                 wo_k[:, nb * 512:(nb + 1) * 512],
                                                 start=st, stop=sp)
                    for t2 in range(2):
                        tti = ch * 2 + t2
                        outsb = tw.tile([128, D], F32)
                        for nb in range(NB):
                            nc.scalar.mul(outsb[:, nb * 512:(nb + 1) * 512],
                                          pso[t2][nb][:], coall[:, tti:tti + 1])
                        nc.sync.dma_start(out_d.ap()[tti * 128:(tti + 1) * 128, :],
                                          outsb[:])

    nc.compile()
    return nc


_NC_CACHE = None
LAST_RESULTS = None


def _get_nc():
    global _NC_CACHE
    if _NC_CACHE is None:
        _NC_CACHE = build_nc()
    return _NC_CACHE


def _quant_weight(w):
    """fla BitLinear ternary weight quant. w [out, in] f32.
    Returns integer-valued bf16 WT [in, out] and the reciprocal scale 1/ws."""
    import jax
    import jax.numpy as jnp

    mean_abs = np.asarray(
        jax.jit(lambda a: jnp.mean(jnp.abs(a)), backend="cpu")(w)
    )
    ws = np.float32(1.0) / np.maximum(mean_abs.astype(np.float32), np.float32(1e-5))
    wq = np.clip(np.round(w * ws), -1.0, 1.0).astype(np.float32)
    return wq.T.copy(), np.float32(1.0) / ws


def kernel(hidden_states, Wi, Wf, Wg, Wo, g_norm_weight):
    nc = _get_nc()

    wiq, rwsi = _quant_weight(np.asarray(Wi))
    wfq, rwsf = _quant_weight(np.asarray(Wf))
    wgq, rwsg = _quant_weight(np.asarray(Wg))
    woq, rwso = _quant_weight(np.asarray(Wo))

    # i/f weights pre-tiled: [mb][p][k][c] = WT[k*128+p, mb*256+c]
    def tile_if(wt):
        return np.ascontiguousarray(
            wt.reshape(KT, 128, MBLK, 256).transpose(2, 1, 0, 3)
        ).astype(ml_dtypes.bfloat16)

    wit = tile_if(wiq)
    wft = tile_if(wfq)
    wgt = wgq.astype(ml_dtypes.bfloat16)
    wot = woq.astype(ml_dtypes.bfloat16)

    idm = np.eye(128, dtype=np.float32)
    gw = np.asarray(g_norm_weight, dtype=np.float32).reshape(1, D)
    x = np.asarray(hidden_states, dtype=np.float32)

    in_maps = []
    for c in range(NCORES):
        b, half = c // 2, c % 2
        rw = np.zeros((128, 5), np.float32)
        rw[:, 0] = rwsi
        rw[:, 1] = rwsf
        rw[:, 2] = -rwsf
        rw[:, 3] = rwsg
        rw[:, 4] = rwso
        in_maps.append({
            "x": np.ascontiguousarray(x[b, half * TPC:(half + 1) * TPC, :]),
            "wit": wit, "wft": wft, "wgt": wgt, "wot": wot,
            "gw": gw, "id128": idm,
            "mask_even": np.full((128, 1), 1.0 - half, np.float32),
            "mask_odd": np.full((128, 1), float(half), np.float32),
            "rws": rw,
        })

    import os
    trace = bool(os.environ.get("HGRN_TRACE"))
    res = run_bass_kernel_spmd(nc, in_maps, list(range(NCORES)), trace=trace)
    global LAST_RESULTS
    LAST_RESULTS = res
    out = np.empty((B, L, D), np.float32)
    for c in range(NCORES):
        b, half = c // 2, c % 2
        out[b, half * TPC:(half + 1) * TPC, :] = res.results[c]["out"]
    return out



# revision 7
# speedup vs baseline: 1.3801x; 1.3801x over previous
"""HGRN BitAttention Trainium2 kernel (8-core SPMD, token-sharded), v2.

Sharding: core c handles batch c//2, sequence half c%2 (1024 tokens).
The HGRN recurrence carry h[1023] crosses the half boundary via two small
pair-AllReduces (m 0-7 and m 8-15) so most of the collective latency hides
under phase-P compute; masks keep the program uniform (SPMD).

BitLinear trick: activations quantize to integers in [-127,127] and weights
to {-1,0,1} - both exact in bf16 - so all four projections run as
exact-integer bf16 matmuls with fp32 PSUM accumulation.

v2 layout strategy: i/f/g projections all run feature-major ([d_out, tok]
PSUM) off one shared transposed-activation operand; the gate
(FusedRMSNormSwishGate) also runs feature-major, using PE ones-matmuls for
the over-feature sums and a gpsimd partition_all_reduce for the over-feature
abs-max, producing quantized o directly in the [d, tok] layout the output
projection consumes as its stationary operand - no transposes. The gate is
token-chunked (4 x 256) so output-projection matmuls pipeline behind it.
"""

import numpy as np
import ml_dtypes

import concourse.bass as bass
import concourse.bacc as bacc
import concourse.mybir as mybir
import concourse.bass_isa as bass_isa
import concourse.tile as tile
from concourse.bass_utils import run_bass_kernel_spmd

F32 = mybir.dt.float32
BF16 = mybir.dt.bfloat16
F16 = mybir.dt.float16
I32 = mybir.dt.int32
AF = mybir.ActivationFunctionType
OP = mybir.AluOpType

B, L, D = 4, 2048, 2048
NCORES = 8
TPC = L // 2          # tokens per core = 1024
NTT = TPC // 128      # 8 token tiles per core
KT = D // 128         # 16 k tiles
MT = D // 128         # 16 m tiles
NCH = 4               # gate/out token chunks of 256
TCH = TPC // NCH      # 256
EPS = 1e-5


def build_nc():
    nc = bacc.Bacc("TRN2", target_bir_lowering=False, debug=False,
                   num_devices=NCORES)

    x_d = nc.dram_tensor("x", [TPC, D], F32, kind="ExternalInput")
    wit_d = nc.dram_tensor("wit", [MT, 128, KT, 128], BF16, kind="ExternalInput")
    wft_d = nc.dram_tensor("wft", [MT, 128, KT, 128], BF16, kind="ExternalInput")
    wgt_d = nc.dram_tensor("wgt", [MT, 128, KT, 128], BF16, kind="ExternalInput")
    wot_d = nc.dram_tensor("wot", [D, D], BF16, kind="ExternalInput")
    gwc_d = nc.dram_tensor("gwc", [128, MT], F32, kind="ExternalInput")
    me_d = nc.dram_tensor("mask_even", [128, 1], F32, kind="ExternalInput")
    mo_d = nc.dram_tensor("mask_odd", [128, 1], F32, kind="ExternalInput")
    rws_d = nc.dram_tensor("rws", [128, 6], F32, kind="ExternalInput")
    out_d = nc.dram_tensor("out", [TPC, D], F32, kind="ExternalOutput")

    with tile.TileContext(nc) as tc:
        with (
            tc.tile_pool(name="const", bufs=1) as cp,
            tc.tile_pool(name="hp", bufs=1) as hp,
            tc.tile_pool(name="dram", bufs=1, space="DRAM") as dram,
        ):
            # ---- constants ----
            me = cp.tile([128, 1], F32)
            nc.sync.dma_start(me[:], me_d.ap())
            mo = cp.tile([128, 1], F32)
            nc.sync.dma_start(mo[:], mo_d.ap())
            gwc = cp.tile([128, MT], F32)
            nc.sync.dma_start(gwc[:], gwc_d.ap())
            rws = cp.tile([128, 6], F32)
            nc.sync.dma_start(rws[:], rws_d.ap())
            rwsi, rwsf, rwsfn, rwsg, rwso, rwso127 = (
                rws[:, i:i + 1] for i in range(6))
            epsb = cp.tile([128, 1], F32)
            nc.vector.memset(epsb[:], EPS)
            zeros = cp.tile([128, 512], F16)
            nc.vector.memset(zeros[:], 0.0)
            ones1 = cp.tile([1, 128], F32)
            nc.vector.memset(ones1[:], 1.0)
            onesc = cp.tile([128, 1], BF16)
            nc.vector.memset(onesc[:], 1.0)

            srec = cp.tile([128, NTT], F32)     # (1/s_x) per token tile col
            S = cp.tile([128, TPC], F32)        # (1/s_x) broadcast, feat-major
            bnd = cp.tile([128, MT], F32)
            hbnd0 = cp.tile([128, MT], F32)     # f32 h boundary of half 0
            carried = cp.tile([128, MT], F32)
            row4k = cp.tile([1, TPC], F32)      # X: 1/s row; G: rstd_g row
            gsqacc = cp.tile([1, TPC], F32)     # sum over features of g^2
            coall = cp.tile([128, NTT], F32)    # per-token out scale, [p,1] x8

            hs = [None] * MT
            fcs = [None] * MT
            g16 = [None] * MT

            fc_ctx = tc.tile_pool(name="fcp", bufs=1)
            fcp = fc_ctx.__enter__()

            xq_ctx = tc.tile_pool(name="xq", bufs=1)
            xqp = xq_ctx.__enter__()
            # two halves of quantized-x, feature-major: [p, k, t(512)]
            xqh = [xqp.tile([128, KT * 512], BF16, name=f"xqh{h}")
                   for h in range(2)]
            xqh3 = [t[:].rearrange("p (k t) -> p k t", k=KT) for t in xqh]

            # ================= Phase X: normalize + quantize x ==============
            with (
                tc.tile_pool(name="xin", bufs=2) as xin,
                tc.tile_pool(name="xwa", bufs=2) as xwa,
                tc.tile_pool(name="xwb", bufs=1) as xwb,
                tc.tile_pool(name="psx", bufs=2, space="PSUM") as psx,
            ):
                for tt in range(NTT):
                    xt = xin.tile([128, D], F32)
                    nc.sync.dma_start(xt[:], x_d.ap()[tt * 128:(tt + 1) * 128, :])
                    scr = xwa.tile([128, D], F32)
                    ssum = xwa.tile([128, 1], F32)
                    nc.scalar.activation(scr[:], xt[:], AF.Square,
                                         accum_out=ssum[:])
                    std = xwa.tile([128, 1], F32)
                    nc.scalar.activation(std[:], ssum[:], AF.Sqrt,
                                         bias=epsb[:], scale=1.0 / D)
                    rstd = xwa.tile([128, 1], F32)
                    nc.vector.reciprocal(rstd[:], std[:])
                    xn = xwb.tile([128, D], F32)
                    nc.scalar.mul(xn[:], xt[:], rstd[:])
                    mxn = xwa.tile([128, 1], F32)
                    nc.vector.tensor_reduce(mxn[:], xn[:], mybir.AxisListType.X,
                                            OP.max, apply_absolute_value=True)
                    nc.vector.tensor_scalar_max(mxn[:], mxn[:], EPS)
                    nc.vector.tensor_scalar_mul(srec[:, tt:tt + 1], mxn[:],
                                                1.0 / 127.0)
                    sst = xwa.tile([128, 1], F32)
                    nc.vector.reciprocal(sst[:], mxn[:])
                    nc.vector.tensor_scalar_mul(sst[:], sst[:], 127.0)
                    qi = xwa.tile([128, D], I32, name="scr")
                    nc.scalar.activation(qi[:], xn[:], AF.Identity, scale=sst[:])
                    qb = xwb.tile([128, D], BF16)
                    nc.vector.tensor_copy(qb[:], qi[:])
                    h, sub = tt // 4, tt % 4
                    nc.sync.dma_start_transpose(
                        xqh3[h][:, :, sub * 128:(sub + 1) * 128], qb[:])

                # S = broadcast of (1/s) to [128, TPC] feature-major
                srd = dram.tile([1, TPC], F32)
                nc.sync.dma_start(
                    srd[:].rearrange("o (t p) -> (o p) t", p=128), srec[:])
                nc.sync.dma_start(row4k[:], srd[:])
                for c in range(2):
                    pS = psx.tile([128, 512], F32)
                    nc.tensor.matmul(pS[:], ones1[:],
                                     row4k[:, c * 512:(c + 1) * 512],
                                     start=True, stop=True)
                    nc.scalar.copy(S[:, c * 512:(c + 1) * 512], pS[:])

            # ====== Phase P: i/f/g projections + scans (feature-major) ======
            with (
                tc.tile_pool(name="wfi", bufs=2) as wfi,
                tc.tile_pool(name="pw", bufs=2) as pw,
                tc.tile_pool(name="pw1", bufs=1) as pw1,
                tc.tile_pool(name="psp", bufs=2, space="PSUM") as psp,
                tc.tile_pool(name="psq", bufs=1, space="PSUM") as psq,
            ):
                for half in range(2):
                    Sh = S[:, half * 512:(half + 1) * 512]
                    gsq_ps = psq.tile([1, 512], F32, name="gsq")
                    for m in range(MT):
                        wi_sb = wfi.tile([128, KT * 128], BF16, name="wi")
                        nc.sync.dma_start(
                            wi_sb[:], wit_d.ap()[m].rearrange("p k c -> p (k c)"))
                        wf_sb = wfi.tile([128, KT * 128], BF16, name="wf")
                        nc.sync.dma_start(
                            wf_sb[:], wft_d.ap()[m].rearrange("p k c -> p (k c)"))
                        wg_sb = wfi.tile([128, KT * 128], BF16, name="wg")
                        nc.sync.dma_start(
                            wg_sb[:], wgt_d.ap()[m].rearrange("p k c -> p (k c)"))
                        psi = psp.tile([128, 512], F32, name="psi")
                        psf = psp.tile([128, 512], F32, name="psf")
                        psg = psp.tile([128, 512], F32, name="psg")
                        for k in range(KT):
                            rhs = xqh[half][:, k * 512:(k + 1) * 512]
                            st, sp = (k == 0), (k == KT - 1)
                            nc.tensor.matmul(psi[:], wi_sb[:, k * 128:(k + 1) * 128],
                                             rhs, start=st, stop=sp)
                            nc.tensor.matmul(psf[:], wf_sb[:, k * 128:(k + 1) * 128],
                                             rhs, start=st, stop=sp)
                            nc.tensor.matmul(psg[:], wg_sb[:, k * 128:(k + 1) * 128],
                                             rhs, start=st, stop=sp)
                        # f path
                        tmpf = pw.tile([128, 512], F32)
                        nc.vector.tensor_tensor(tmpf[:], psf[:], Sh, OP.mult)
                        G = pw.tile([128, 512], F32)
                        nc.scalar.activation(G[:], tmpf[:], AF.Sigmoid,
                                             scale=rwsfn)
                        F = pw.tile([128, 512], F32)
                        nc.scalar.activation(F[:], tmpf[:], AF.Sigmoid,
                                             scale=rwsf)
                        # i path
                        tmpi = pw.tile([128, 512], F32)
                        nc.vector.tensor_tensor(tmpi[:], psi[:], Sh, OP.mult)
                        sgi = pw1.tile([128, 512], F32)
                        nc.scalar.activation(sgi[:], tmpi[:], AF.Sigmoid,
                                             scale=rwsi)
                        SiL2 = pw.tile([128, 512], F32)
                        nc.vector.scalar_tensor_tensor(SiL2[:], tmpi[:], rwsi,
                                                       sgi[:], OP.mult, OP.mult)
                        Iin = pw.tile([128, 512], F32)
                        nc.vector.tensor_tensor(Iin[:], SiL2[:], G[:], OP.mult)
                        # g path (store dequantized g as f16)
                        if half == 0:
                            g16[m] = hp.tile([128, TPC], F16, name=f"g_{m}")
                            hs[m] = hp.tile([128, TPC], BF16, name=f"h_{m}")
                            fcs[m] = fcp.tile([128, TPC], F16, name=f"fc_{m}")
                        gsl = g16[m][:, half * 512:(half + 1) * 512]
                        nc.vector.scalar_tensor_tensor(gsl, psg[:], rwsg, Sh,
                                                       OP.mult, OP.mult)
                        g2 = pw.tile([128, 512], BF16)
                        nc.scalar.activation(g2[:], gsl, AF.Square)
                        nc.tensor.matmul(gsq_ps[:], onesc[:], g2[:],
                                         start=(m == 0), stop=(m == MT - 1))
                        # scans (f32 transient keeps an exact carry boundary)
                        htmp = pw1.tile([128, 512], F32)
                        if half == 0:
                            nc.vector.tensor_tensor_scan(
                                htmp[:], F[:], Iin[:], 0.0, OP.mult, OP.add)
                            nc.vector.tensor_copy(hbnd0[:, m:m + 1],
                                                  htmp[:, 511:512])
                            nc.vector.tensor_tensor_scan(
                                fcs[m][:, 0:512], F[:], zeros[:], 1.0,
                                OP.mult, OP.add)
                        else:
                            nc.vector.tensor_tensor_scan(
                                htmp[:], F[:], Iin[:], hbnd0[:, m:m + 1],
                                OP.mult, OP.add)
                            nc.vector.tensor_copy(bnd[:, m:m + 1],
                                                  htmp[:, 511:512])
                            nc.vector.tensor_tensor_scan(
                                fcs[m][:, 512:1024], F[:], zeros[:],
                                fcs[m][:, 511:512], OP.mult, OP.add)
                        nc.vector.tensor_copy(
                            hs[m][:, half * 512:(half + 1) * 512], htmp[:])

                        # carry exchange, split in two to hide latency
                        if half == 1 and m in (MT // 2 - 1, MT - 1):
                            lo = 0 if m < MT // 2 else MT // 2
                            hi = lo + MT // 2
                            bnd2 = cp.tile([128, MT // 2], F32,
                                           name=f"bnd2_{lo}")
                            nc.vector.tensor_scalar_mul(
                                bnd2[:], bnd[:, lo:hi], me[:])
                            cin = dram.tile([128, MT // 2], F32)
                            cout = dram.tile([128, MT // 2], F32)
                            nc.sync.dma_start(cin[:], bnd2[:])
                            nc.gpsimd.collective_compute(
                                "AllReduce", OP.add,
                                replica_groups=[[0, 1], [2, 3], [4, 5], [6, 7]],
                                ins=[cin.opt()], outs=[cout.opt()],
                            )
                            csb = cp.tile([128, MT // 2], F32,
                                          name=f"csb_{lo}")
                            nc.sync.dma_start(csb[:], cout[:])
                            nc.vector.tensor_scalar_mul(
                                carried[:, lo:hi], csb[:], mo[:])
                    # stash this half's g^2 column sums before psum reuse
                    nc.scalar.copy(gsqacc[:, half * 512:(half + 1) * 512],
                                   gsq_ps[:])

            xq_ctx.__exit__(None, None, None)

            # carry fixup in place (bf16), then retire the cumprod tiles
            for m in range(MT):
                nc.vector.scalar_tensor_tensor(
                    hs[m][:], fcs[m][:], carried[:, m:m + 1], hs[m][:],
                    OP.mult, OP.add)
            fc_ctx.__exit__(None, None, None)

            # ============ Phase G+T: gate + output projection ==============
            with (
                tc.tile_pool(name="gw2", bufs=2) as gp,
                tc.tile_pool(name="gw1", bufs=1) as gp1,
                tc.tile_pool(name="oqt", bufs=1) as oqtp,
                tc.tile_pool(name="wst", bufs=3) as wst,
                tc.tile_pool(name="tout", bufs=1) as top_,
                tc.tile_pool(name="psg2", bufs=1, space="PSUM") as psg2,
                tc.tile_pool(name="psb", bufs=1, space="PSUM") as psb,
                tc.tile_pool(name="pst", bufs=5, space="PSUM") as pst,
            ):
                # rstd_g row (reuses the X-phase row slot) + broadcast
                nc.scalar.activation(row4k[:], gsqacc[:], AF.Sqrt,
                                     bias=epsb[0:1, :], scale=1.0 / D)
                nc.vector.reciprocal(row4k[:], row4k[:])
                rstdg_b = gp1.tile([128, TPC], F32, name="rstdg_b")
                for c in range(2):
                    pS = psb.tile([128, 512], F32, name="bc")
                    nc.tensor.matmul(pS[:], ones1[:],
                                     row4k[:, c * 512:(c + 1) * 512],
                                     start=True, stop=True)
                    nc.scalar.copy(rstdg_b[:, c * 512:(c + 1) * 512], pS[:])

                oqb = [oqtp.tile([128, TPC], BF16, name=f"oq_{m}")
                       for m in range(MT)]

                for ch in range(NCH):
                    tsl = slice(ch * TCH, (ch + 1) * TCH)
                    # --- gate sweep over feature tiles ---
                    tmps = []
                    macc = gp1.tile([128, TCH], F32, name="macc")
                    osq_ps = psg2.tile([1, TCH], F32, name="osq")
                    for m in range(MT):
                        hsw = gp.tile([128, TCH], F32, name="hsw")
                        nc.scalar.activation(hsw[:], hs[m][:, tsl], AF.Silu)
                        tmp = gp.tile([128, TCH], F32, name=f"tmp_{m}")
                        nc.vector.scalar_tensor_tensor(
                            tmp[:], g16[m][:, tsl], gwc[:, m:m + 1], hsw[:],
                            OP.mult, OP.mult)
                        tmps.append(tmp)
                        tmp2 = gp.tile([128, TCH], F32, name="tmp2")
                        nc.scalar.activation(tmp2[:], tmp[:], AF.Square)
                        if m == 0:
                            nc.vector.tensor_copy(macc[:], tmp2[:])
                        else:
                            nc.vector.tensor_tensor(macc[:], macc[:], tmp2[:],
                                                    OP.max)
                        tmp2b = gp.tile([128, TCH], BF16, name="tmp2b")
                        nc.vector.tensor_copy(tmp2b[:], tmp2[:])
                        nc.tensor.matmul(osq_ps[:], onesc[:], tmp2b[:],
                                         start=(m == 0), stop=(m == MT - 1))
                    # --- per-token scale rows ---
                    pmx = gp1.tile([128, TCH], F32, name="pmx")
                    nc.gpsimd.partition_all_reduce(pmx[:], macc[:], 128,
                                                   bass_isa.ReduceOp.max)
                    mxt = gp1.tile([1, TCH], F32, name="mxt")
                    nc.scalar.activation(mxt[:], pmx[0:1, :], AF.Sqrt)
                    mo2 = gp1.tile([1, TCH], F32, name="mo2")
                    nc.vector.tensor_tensor(mo2[:], osq_ps[:],
                                            row4k[0:1, tsl], OP.mult)
                    nc.vector.tensor_tensor(mo2[:], mo2[:],
                                            row4k[0:1, tsl], OP.mult)
                    rso = gp1.tile([1, TCH], F32, name="rso")
                    nc.scalar.activation(rso[:], mo2[:], AF.Sqrt,
                                         bias=epsb[0:1, :], scale=1.0 / D)
                    nc.vector.reciprocal(rso[:], rso[:])
                    mxn = gp1.tile([1, TCH], F32, name="mxn")
                    nc.vector.tensor_tensor(mxn[:], mxt[:],
                                            row4k[0:1, tsl], OP.mult)
                    nc.vector.tensor_tensor(mxn[:], mxn[:], rso[:], OP.mult)
                    nc.vector.tensor_scalar_max(mxn[:], mxn[:], EPS)
                    # output scale row -> [p,1] columns via PE transpose
                    osc = gp1.tile([1, TCH], F32, name="osc")
                    nc.vector.tensor_scalar_mul(osc[:], mxn[:],
                                                rwso127[0:1, :])
                    for t2 in range(2):
                        tti = ch * 2 + t2
                        ptc = psb.tile([128, 1], F32, name="ptc")
                        nc.tensor.matmul(ptc[:],
                                         osc[:, t2 * 128:(t2 + 1) * 128],
                                         ones1[:, 0:1], is_transpose=True,
                                         start=True, stop=True)
                        nc.scalar.copy(coall[:, tti:tti + 1], ptc[:])
                    # qmul = (127 / mxn) * rstd_o * rstd_g
                    qm = gp1.tile([1, TCH], F32, name="qm")
                    nc.vector.reciprocal(qm[:], mxn[:])
                    nc.vector.tensor_scalar_mul(qm[:], qm[:], 127.0)
                    nc.vector.tensor_tensor(qm[:], qm[:], rso[:], OP.mult)
                    nc.vector.tensor_tensor(qm[:], qm[:], row4k[0:1, tsl],
                                            OP.mult)
                    qmb = gp.tile([128, TCH], F32, name="qmb")
                    pq = psb.tile([128, TCH], F32, name="bc")
                    nc.tensor.matmul(pq[:], ones1[:], qm[:],
                                     start=True, stop=True)
                    nc.scalar.copy(qmb[:], pq[:])
                    # --- quantize o (feature-major: already the lhsT layout) ---
                    for m in range(MT):
                        oqi = gp.tile([128, TCH], I32, name="oqi")
                        nc.vector.tensor_tensor(oqi[:], tmps[m][:], qmb[:],
                                                OP.mult)
                        nc.scalar.copy(oqb[m][:, tsl], oqi[:])
                    # --- output projection for this chunk's 2 token tiles ---
                    outsb = [top_.tile([128, D], F32, name=f"outsb_{t2}")
                             for t2 in range(2)]
                    for nbh in range(2):
                        po = [pst.tile([128, 512], F32, name="po")
                              for _ in range(4)]
                        for k in range(KT):
                            wo_k = wst.tile([128, 1024], BF16, name="wo_k")
                            nc.sync.dma_start(
                                wo_k[:],
                                wot_d.ap()[k * 128:(k + 1) * 128,
                                           nbh * 1024:(nbh + 1) * 1024])
                            st, sp = (k == 0), (k == KT - 1)
                            for t2 in range(2):
                                lhsT = oqb[k][:, (ch * 2 + t2) * 128:
                                              (ch * 2 + t2 + 1) * 128]
                                for n2 in range(2):
                                    nc.tensor.matmul(
                                        po[t2 * 2 + n2][:], lhsT,
                                        wo_k[:, n2 * 512:(n2 + 1) * 512],
                                        start=st, stop=sp)
                        for t2 in range(2):
                            tti = ch * 2 + t2
                            for n2 in range(2):
                                nc.scalar.mul(
                                    outsb[t2][:, nbh * 1024 + n2 * 512:
                                              nbh * 1024 + (n2 + 1) * 512],
                                    po[t2 * 2 + n2][:],
                                    coall[:, tti:tti + 1])
                    for t2 in range(2):
                        tti = ch * 2 + t2
                        nc.sync.dma_start(
                            out_d.ap()[tti * 128:(tti + 1) * 128, :],
                            outsb[t2][:])

    nc.compile()
    return nc


_NC_CACHE = None
LAST_RESULTS = None


def _get_nc():
    global _NC_CACHE
    if _NC_CACHE is None:
        _NC_CACHE = build_nc()
    return _NC_CACHE


def _quant_weight(w):
    """fla BitLinear ternary weight quant. w [out, in] f32.
    Returns integer-valued f32 WT [in, out] and the reciprocal scale 1/ws."""
    import jax
    import jax.numpy as jnp

    mean_abs = np.asarray(
        jax.jit(lambda a: jnp.mean(jnp.abs(a)), backend="cpu")(w)
    )
    ws = np.float32(1.0) / np.maximum(mean_abs.astype(np.float32),
                                      np.float32(1e-5))
    wq = np.clip(np.round(w * ws), -1.0, 1.0).astype(np.float32)
    return wq.T.copy(), np.float32(1.0) / ws


def kernel(hidden_states, Wi, Wf, Wg, Wo, g_norm_weight):
    nc = _get_nc()

    wiq, rwsi = _quant_weight(np.asarray(Wi))
    wfq, rwsf = _quant_weight(np.asarray(Wf))
    wgq, rwsg = _quant_weight(np.asarray(Wg))
    woq, rwso = _quant_weight(np.asarray(Wo))

    # lhsT tiles pre-tiled per m: [m][p][k][c] = WT[k*128+p, m*128+c]
    def tile_lhs(wt):
        return np.ascontiguousarray(
            wt.reshape(KT, 128, MT, 128).transpose(2, 1, 0, 3)
        ).astype(ml_dtypes.bfloat16)

    wit = tile_lhs(wiq)
    wft = tile_lhs(wfq)
    wgt = tile_lhs(wgq)
    wot = woq.astype(ml_dtypes.bfloat16)

    gw = np.asarray(g_norm_weight, dtype=np.float32)
    gwc = np.ascontiguousarray(gw.reshape(MT, 128).T)
    x = np.asarray(hidden_states, dtype=np.float32)

    in_maps = []
    for c in range(NCORES):
        b, half = c // 2, c % 2
        rw = np.zeros((128, 6), np.float32)
        rw[:, 0] = rwsi
        rw[:, 1] = rwsf
        rw[:, 2] = -rwsf
        rw[:, 3] = rwsg
        rw[:, 4] = rwso
        rw[:, 5] = rwso / np.float32(127.0)
        in_maps.append({
            "x": np.ascontiguousarray(x[b, half * TPC:(half + 1) * TPC, :]),
            "wit": wit, "wft": wft, "wgt": wgt, "wot": wot,
            "gwc": gwc,
            "mask_even": np.full((128, 1), 1.0 - half, np.float32),
            "mask_odd": np.full((128, 1), float(half), np.float32),
            "rws": rw,
        })

    import os
    trace = bool(os.environ.get("HGRN_TRACE"))
    res = run_bass_kernel_spmd(nc, in_maps, list(range(NCORES)), trace=trace)
    global LAST_RESULTS
    LAST_RESULTS = res
    out = np.empty((B, L, D), np.float32)
    for c in range(NCORES):
        b, half = c // 2, c % 2
        out[b, half * TPC:(half + 1) * TPC, :] = res.results[c]["out"]
    return out


# revision 16
# speedup vs baseline: 1.3915x; 1.0083x over previous
"""HGRN BitAttention Trainium2 kernel (8-core SPMD, token-sharded), v2.

Sharding: core c handles batch c//2, sequence half c%2 (1024 tokens).
The HGRN recurrence carry h[1023] crosses the half boundary via two small
pair-AllReduces (m 0-7 and m 8-15) so most of the collective latency hides
under phase-P compute; masks keep the program uniform (SPMD).

BitLinear trick: activations quantize to integers in [-127,127] and weights
to {-1,0,1} - both exact in bf16 - so all four projections run as
exact-integer bf16 matmuls with fp32 PSUM accumulation.

v2 layout strategy: i/f/g projections all run feature-major ([d_out, tok]
PSUM) off one shared transposed-activation operand; the gate
(FusedRMSNormSwishGate) also runs feature-major, using PE ones-matmuls for
the over-feature sums and a gpsimd partition_all_reduce for the over-feature
abs-max, producing quantized o directly in the [d, tok] layout the output
projection consumes as its stationary operand - no transposes. The gate is
token-chunked (4 x 256) so output-projection matmuls pipeline behind it.
"""

import numpy as np
import ml_dtypes

import concourse.bass as bass
import concourse.bacc as bacc
import concourse.mybir as mybir
import concourse.bass_isa as bass_isa
import concourse.tile as tile
from concourse.bass_utils import run_bass_kernel_spmd

F32 = mybir.dt.float32
BF16 = mybir.dt.bfloat16
F16 = mybir.dt.float16
I32 = mybir.dt.int32
AF = mybir.ActivationFunctionType
OP = mybir.AluOpType

B, L, D = 4, 2048, 2048
NCORES = 8
TPC = L // 2          # tokens per core = 1024
NTT = TPC // 128      # 8 token tiles per core
KT = D // 128         # 16 k tiles
MT = D // 128         # 16 m tiles
NCH = 4               # gate/out token chunks of 256
TCH = TPC // NCH      # 256
EPS = 1e-5


def build_nc():
    nc = bacc.Bacc("TRN2", target_bir_lowering=False, debug=False,
                   num_devices=NCORES)

    x_d = nc.dram_tensor("x", [TPC, D], F32, kind="ExternalInput")
    wit_d = nc.dram_tensor("wit", [MT, 128, KT, 128], BF16, kind="ExternalInput")
    wft_d = nc.dram_tensor("wft", [MT, 128, KT, 128], BF16, kind="ExternalInput")
    wgt_d = nc.dram_tensor("wgt", [MT, 128, KT, 128], BF16, kind="ExternalInput")
    wot_d = nc.dram_tensor("wot", [D, D], BF16, kind="ExternalInput")
    gwc_d = nc.dram_tensor("gwc", [128, MT], F32, kind="ExternalInput")
    me_d = nc.dram_tensor("mask_even", [128, 1], F32, kind="ExternalInput")
    mo_d = nc.dram_tensor("mask_odd", [128, 1], F32, kind="ExternalInput")
    rws_d = nc.dram_tensor("rws", [128, 6], F32, kind="ExternalInput")
    out_d = nc.dram_tensor("out", [TPC, D], F32, kind="ExternalOutput")

    with tile.TileContext(nc) as tc:
        with (
            tc.tile_pool(name="const", bufs=1) as cp,
            tc.tile_pool(name="hp", bufs=1) as hp,
            tc.tile_pool(name="dram", bufs=1, space="DRAM") as dram,
        ):
            # ---- constants ----
            me = cp.tile([128, 1], F32)
            nc.sync.dma_start(me[:], me_d.ap())
            mo = cp.tile([128, 1], F32)
            nc.sync.dma_start(mo[:], mo_d.ap())
            gwc = cp.tile([128, MT], F32)
            nc.sync.dma_start(gwc[:], gwc_d.ap())
            rws = cp.tile([128, 6], F32)
            nc.sync.dma_start(rws[:], rws_d.ap())
            rwsi, rwsf, rwsfn, rwsg, rwso, rwso127 = (
                rws[:, i:i + 1] for i in range(6))
            epsb = cp.tile([128, 1], F32)
            nc.vector.memset(epsb[:], EPS)
            zeros = cp.tile([128, 512], F16)
            nc.vector.memset(zeros[:], 0.0)
            ones1 = cp.tile([1, 128], F32)
            nc.vector.memset(ones1[:], 1.0)
            onesc = cp.tile([128, 1], BF16)
            nc.vector.memset(onesc[:], 1.0)

            srec = cp.tile([128, NTT], F32)     # (1/s_x) per token tile col
            S = cp.tile([128, TPC], F32)        # (1/s_x) broadcast, feat-major
            bnd = cp.tile([128, MT], F32)
            hbnd0 = cp.tile([128, MT], F32)     # f32 h boundary of half 0
            carried = cp.tile([128, MT], F32)
            row4k = cp.tile([1, TPC], F32)      # X: 1/s row; G: rstd_g row
            gsqacc = cp.tile([1, TPC], F32)     # sum over features of g^2
            coall = cp.tile([128, NTT], F32)    # per-token out scale, [p,1] x8

            hs = [None] * MT
            fcs = [None] * MT
            g16 = [None] * MT

            fc_ctx = tc.tile_pool(name="fcp", bufs=1)
            fcp = fc_ctx.__enter__()

            xq_ctx = tc.tile_pool(name="xq", bufs=1)
            xqp = xq_ctx.__enter__()
            # two halves of quantized-x, feature-major: [p, k, t(512)]
            xqh = [xqp.tile([128, KT * 512], BF16, name=f"xqh{h}")
                   for h in range(2)]
            xqh3 = [t[:].rearrange("p (k t) -> p k t", k=KT) for t in xqh]

            # ================= Phase X: normalize + quantize x ==============
            with (
                tc.tile_pool(name="xin", bufs=2) as xin,
                tc.tile_pool(name="xwa", bufs=2) as xwa,
                tc.tile_pool(name="xwb", bufs=1) as xwb,
                tc.tile_pool(name="psx", bufs=2, space="PSUM") as psx,
            ):
                for tt in range(NTT):
                    xt = xin.tile([128, D], F32)
                    nc.sync.dma_start(xt[:], x_d.ap()[tt * 128:(tt + 1) * 128, :])
                    scr = xwa.tile([128, D], F32)
                    ssum = xwa.tile([128, 1], F32)
                    nc.scalar.activation(scr[:], xt[:], AF.Square,
                                         accum_out=ssum[:])
                    std = xwa.tile([128, 1], F32)
                    nc.scalar.activation(std[:], ssum[:], AF.Sqrt,
                                         bias=epsb[:], scale=1.0 / D)
                    rstd = xwa.tile([128, 1], F32)
                    nc.vector.reciprocal(rstd[:], std[:])
                    xn = xwb.tile([128, D], F32)
                    nc.scalar.mul(xn[:], xt[:], rstd[:])
                    mxn = xwa.tile([128, 1], F32)
                    nc.vector.tensor_reduce(mxn[:], xn[:], mybir.AxisListType.X,
                                            OP.max, apply_absolute_value=True)
                    nc.vector.tensor_scalar_max(mxn[:], mxn[:], EPS)
                    nc.vector.tensor_scalar_mul(srec[:, tt:tt + 1], mxn[:],
                                                1.0 / 127.0)
                    sst = xwa.tile([128, 1], F32)
                    nc.vector.reciprocal(sst[:], mxn[:])
                    nc.vector.tensor_scalar_mul(sst[:], sst[:], 127.0)
                    qi = xwa.tile([128, D], I32, name="scr")
                    nc.scalar.activation(qi[:], xn[:], AF.Identity, scale=sst[:])
                    qb = xwb.tile([128, D], BF16)
                    nc.vector.tensor_copy(qb[:], qi[:])
                    h, sub = tt // 4, tt % 4
                    nc.sync.dma_start_transpose(
                        xqh3[h][:, :, sub * 128:(sub + 1) * 128], qb[:])

                # S = broadcast of (1/s) to [128, TPC] feature-major
                srd = dram.tile([1, TPC], F32)
                nc.sync.dma_start(
                    srd[:].rearrange("o (t p) -> (o p) t", p=128), srec[:])
                nc.sync.dma_start(row4k[:], srd[:])
                for c in range(2):
                    pS = psx.tile([128, 512], F32)
                    nc.tensor.matmul(pS[:], ones1[:],
                                     row4k[:, c * 512:(c + 1) * 512],
                                     start=True, stop=True)
                    nc.scalar.copy(S[:, c * 512:(c + 1) * 512], pS[:])

            # ====== Phase P: i/f/g projections + scans (feature-major) ======
            with (
                tc.tile_pool(name="wfi", bufs=2) as wfi,
                tc.tile_pool(name="pw", bufs=2) as pw,
                tc.tile_pool(name="pw1", bufs=1) as pw1,
                tc.tile_pool(name="psp", bufs=2, space="PSUM") as psp,
                tc.tile_pool(name="psq", bufs=1, space="PSUM") as psq,
            ):
                for half in range(2):
                    Sh = S[:, half * 512:(half + 1) * 512]
                    gsq_ps = psq.tile([1, 512], F32, name="gsq")
                    # half 1 runs m 8-15 first so its carry exchange issues
                    # early and m 0-7's exchange is the only one at phase end
                    morder = (list(range(MT)) if half == 0 else
                              list(range(MT // 2, MT)) + list(range(MT // 2)))
                    for mi, m in enumerate(morder):
                        # weight loads ride the ACT HWDGE ring so they are
                        # not FIFO-serialized behind phase-X transfers
                        wi_sb = wfi.tile([128, KT * 128], BF16, name="wi")
                        nc.scalar.dma_start(
                            wi_sb[:], wit_d.ap()[m].rearrange("p k c -> p (k c)"))
                        wf_sb = wfi.tile([128, KT * 128], BF16, name="wf")
                        nc.scalar.dma_start(
                            wf_sb[:], wft_d.ap()[m].rearrange("p k c -> p (k c)"))
                        wg_sb = wfi.tile([128, KT * 128], BF16, name="wg")
                        nc.scalar.dma_start(
                            wg_sb[:], wgt_d.ap()[m].rearrange("p k c -> p (k c)"))
                        psi = psp.tile([128, 512], F32, name="psi")
                        psf = psp.tile([128, 512], F32, name="psf")
                        psg = psp.tile([128, 512], F32, name="psg")
                        for k in range(KT):
                            rhs = xqh[half][:, k * 512:(k + 1) * 512]
                            st, sp = (k == 0), (k == KT - 1)
                            nc.tensor.matmul(psi[:], wi_sb[:, k * 128:(k + 1) * 128],
                                             rhs, start=st, stop=sp)
                            nc.tensor.matmul(psf[:], wf_sb[:, k * 128:(k + 1) * 128],
                                             rhs, start=st, stop=sp)
                            nc.tensor.matmul(psg[:], wg_sb[:, k * 128:(k + 1) * 128],
                                             rhs, start=st, stop=sp)
                        # f path
                        tmpf = pw.tile([128, 512], F32)
                        nc.vector.tensor_tensor(tmpf[:], psf[:], Sh, OP.mult)
                        G = pw.tile([128, 512], F32)
                        nc.scalar.activation(G[:], tmpf[:], AF.Sigmoid,
                                             scale=rwsfn)
                        F = pw.tile([128, 512], F32)
                        nc.scalar.activation(F[:], tmpf[:], AF.Sigmoid,
                                             scale=rwsf)
                        # i path
                        tmpi = pw.tile([128, 512], F32)
                        nc.vector.tensor_tensor(tmpi[:], psi[:], Sh, OP.mult)
                        sgi = pw1.tile([128, 512], F32)
                        nc.scalar.activation(sgi[:], tmpi[:], AF.Sigmoid,
                                             scale=rwsi)
                        SiL2 = pw.tile([128, 512], F32)
                        nc.vector.scalar_tensor_tensor(SiL2[:], tmpi[:], rwsi,
                                                       sgi[:], OP.mult, OP.mult)
                        Iin = pw.tile([128, 512], F32)
                        nc.vector.tensor_tensor(Iin[:], SiL2[:], G[:], OP.mult)
                        # g path (store dequantized g as f16)
                        if half == 0:
                            g16[m] = hp.tile([128, TPC], F16, name=f"g_{m}")
                            hs[m] = hp.tile([128, TPC], BF16, name=f"h_{m}")
                            fcs[m] = fcp.tile([128, TPC], F16, name=f"fc_{m}")
                        gsl = g16[m][:, half * 512:(half + 1) * 512]
                        nc.vector.scalar_tensor_tensor(gsl, psg[:], rwsg, Sh,
                                                       OP.mult, OP.mult)
                        g2 = pw.tile([128, 512], BF16)
                        nc.scalar.activation(g2[:], gsl, AF.Square)
                        nc.tensor.matmul(gsq_ps[:], onesc[:], g2[:],
                                         start=(mi == 0), stop=(mi == MT - 1))
                        # scans (f32 transient keeps an exact carry boundary)
                        htmp = pw1.tile([128, 512], F32)
                        if half == 0:
                            nc.vector.tensor_tensor_scan(
                                htmp[:], F[:], Iin[:], 0.0, OP.mult, OP.add)
                            nc.vector.tensor_copy(hbnd0[:, m:m + 1],
                                                  htmp[:, 511:512])
                            nc.vector.tensor_tensor_scan(
                                fcs[m][:, 0:512], F[:], zeros[:], 1.0,
                                OP.mult, OP.add)
                        else:
                            nc.vector.tensor_tensor_scan(
                                htmp[:], F[:], Iin[:], hbnd0[:, m:m + 1],
                                OP.mult, OP.add)
                            nc.vector.tensor_copy(bnd[:, m:m + 1],
                                                  htmp[:, 511:512])
                            nc.vector.tensor_tensor_scan(
                                fcs[m][:, 512:1024], F[:], zeros[:],
                                fcs[m][:, 511:512], OP.mult, OP.add)
                        nc.vector.tensor_copy(
                            hs[m][:, half * 512:(half + 1) * 512], htmp[:])

                        # carry exchange, split in two to hide latency
                        if half == 1 and mi in (MT // 2 - 1, MT - 1):
                            lo = MT // 2 if mi < MT // 2 else 0
                            hi = lo + MT // 2
                            bnd2 = cp.tile([128, MT // 2], F32,
                                           name=f"bnd2_{lo}")
                            nc.vector.tensor_scalar_mul(
                                bnd2[:], bnd[:, lo:hi], me[:])
                            cin = dram.tile([128, MT // 2], F32)
                            cout = dram.tile([128, MT // 2], F32)
                            nc.sync.dma_start(cin[:], bnd2[:])
                            nc.gpsimd.collective_compute(
                                "AllReduce", OP.add,
                                replica_groups=[[0, 1], [2, 3], [4, 5], [6, 7]],
                                ins=[cin.opt()], outs=[cout.opt()],
                            )
                            csb = cp.tile([128, MT // 2], F32,
                                          name=f"csb_{lo}")
                            nc.sync.dma_start(csb[:], cout[:])
                            nc.vector.tensor_scalar_mul(
                                carried[:, lo:hi], csb[:], mo[:])
                    # stash this half's g^2 column sums before psum reuse
                    nc.scalar.copy(gsqacc[:, half * 512:(half + 1) * 512],
                                   gsq_ps[:])

            xq_ctx.__exit__(None, None, None)

            # carry fixup in place (bf16), then retire the cumprod tiles
            MORDER = list(range(MT // 2, MT)) + list(range(MT // 2))
            for m in MORDER:
                nc.vector.scalar_tensor_tensor(
                    hs[m][:], fcs[m][:], carried[:, m:m + 1], hs[m][:],
                    OP.mult, OP.add)
            fc_ctx.__exit__(None, None, None)

            # ============ Phase G+T: gate + output projection ==============
            with (
                tc.tile_pool(name="gw2", bufs=2) as gp,
                tc.tile_pool(name="gw1", bufs=1) as gp1,
                tc.tile_pool(name="tpp", bufs=1) as tpp,
                tc.tile_pool(name="oqt", bufs=1) as oqtp,
                tc.tile_pool(name="wst", bufs=3) as wst,
                tc.tile_pool(name="tout", bufs=1) as top_,
                tc.tile_pool(name="psg2", bufs=1, space="PSUM") as psg2,
                tc.tile_pool(name="psb", bufs=1, space="PSUM") as psb,
                tc.tile_pool(name="pst", bufs=5, space="PSUM") as pst,
            ):
                # rstd_g row (reuses the X-phase row slot) + broadcast
                nc.scalar.activation(row4k[:], gsqacc[:], AF.Sqrt,
                                     bias=epsb[0:1, :], scale=1.0 / D)
                nc.vector.reciprocal(row4k[:], row4k[:])
                rstdg_b = gp1.tile([128, TPC], F32, name="rstdg_b")
                for c in range(2):
                    pS = psb.tile([128, 512], F32, name="bc")
                    nc.tensor.matmul(pS[:], ones1[:],
                                     row4k[:, c * 512:(c + 1) * 512],
                                     start=True, stop=True)
                    nc.scalar.copy(rstdg_b[:, c * 512:(c + 1) * 512], pS[:])

                oqb = [oqtp.tile([128, TPC], BF16, name=f"oq_{m}")
                       for m in range(MT)]

                for ch in range(NCH):
                    tsl = slice(ch * TCH, (ch + 1) * TCH)
                    # --- gate sweep over feature tiles ---
                    tmps = {}
                    macc = gp1.tile([128, TCH], F32, name="macc")
                    osq_ps = psg2.tile([1, TCH], F32, name="osq")
                    for mi, m in enumerate(MORDER):
                        hsw = gp.tile([128, TCH], F32, name="hsw")
                        nc.scalar.activation(hsw[:], hs[m][:, tsl], AF.Silu)
                        tmp = tpp.tile([128, TCH], F32, name=f"tmp_{m}")
                        nc.vector.scalar_tensor_tensor(
                            tmp[:], g16[m][:, tsl], gwc[:, m:m + 1], hsw[:],
                            OP.mult, OP.mult)
                        tmps[m] = tmp
                        tmp2 = gp.tile([128, TCH], F32, name="tmp2")
                        nc.scalar.activation(tmp2[:], tmp[:], AF.Square)
                        if mi == 0:
                            nc.vector.tensor_copy(macc[:], tmp2[:])
                        else:
                            nc.vector.tensor_tensor(macc[:], macc[:], tmp2[:],
                                                    OP.max)
                        tmp2b = gp.tile([128, TCH], BF16, name="tmp2b")
                        nc.vector.tensor_copy(tmp2b[:], tmp2[:])
                        nc.tensor.matmul(osq_ps[:], onesc[:], tmp2b[:],
                                         start=(mi == 0), stop=(mi == MT - 1))
                    # --- per-token scale rows ---
                    pmx = gp1.tile([128, TCH], F32, name="pmx")
                    nc.gpsimd.partition_all_reduce(pmx[:], macc[:], 128,
                                                   bass_isa.ReduceOp.max)
                    mxt = gp1.tile([1, TCH], F32, name="mxt")
                    nc.scalar.activation(mxt[:], pmx[0:1, :], AF.Sqrt)
                    mo2 = gp1.tile([1, TCH], F32, name="mo2")
                    nc.vector.tensor_tensor(mo2[:], osq_ps[:],
                                            row4k[0:1, tsl], OP.mult)
                    nc.vector.tensor_tensor(mo2[:], mo2[:],
                                            row4k[0:1, tsl], OP.mult)
                    rso = gp1.tile([1, TCH], F32, name="rso")
                    nc.scalar.activation(rso[:], mo2[:], AF.Sqrt,
                                         bias=epsb[0:1, :], scale=1.0 / D)
                    nc.vector.reciprocal(rso[:], rso[:])
                    mxn = gp1.tile([1, TCH], F32, name="mxn")
                    nc.vector.tensor_tensor(mxn[:], mxt[:],
                                            row4k[0:1, tsl], OP.mult)
                    nc.vector.tensor_tensor(mxn[:], mxn[:], rso[:], OP.mult)
                    nc.vector.tensor_scalar_max(mxn[:], mxn[:], EPS)
                    # output scale row -> [p,1] columns via PE transpose
                    osc = gp1.tile([1, TCH], F32, name="osc")
                    nc.vector.tensor_scalar_mul(osc[:], mxn[:],
                                                rwso127[0:1, :])
                    for t2 in range(2):
                        tti = ch * 2 + t2
                        ptc = psb.tile([128, 1], F32, name="ptc")
                        nc.tensor.matmul(ptc[:],
                                         osc[:, t2 * 128:(t2 + 1) * 128],
                                         ones1[:, 0:1], is_transpose=True,
                                         start=True, stop=True)
                        nc.scalar.copy(coall[:, tti:tti + 1], ptc[:])
                    # qmul = (127 / mxn) * rstd_o * rstd_g
                    qm = gp1.tile([1, TCH], F32, name="qm")
                    nc.vector.reciprocal(qm[:], mxn[:])
                    nc.vector.tensor_scalar_mul(qm[:], qm[:], 127.0)
                    nc.vector.tensor_tensor(qm[:], qm[:], rso[:], OP.mult)
                    nc.vector.tensor_tensor(qm[:], qm[:], row4k[0:1, tsl],
                                            OP.mult)
                    qmb = gp.tile([128, TCH], F32, name="qmb")
                    pq = psb.tile([128, TCH], F32, name="bc")
                    nc.tensor.matmul(pq[:], ones1[:], qm[:],
                                     start=True, stop=True)
                    nc.scalar.copy(qmb[:], pq[:])
                    # --- quantize o (feature-major: already the lhsT layout) ---
                    for m in MORDER:
                        oqi = gp.tile([128, TCH], I32, name="oqi")
                        nc.vector.tensor_tensor(oqi[:], tmps[m][:], qmb[:],
                                                OP.mult)
                        nc.scalar.copy(oqb[m][:, tsl], oqi[:])
                    # --- output projection for this chunk's 2 token tiles ---
                    if ch == 0:
                        # cache the low half of Wo in SBUF for all chunks
                        wor = gp1.tile([128, KT * 1024], BF16, name="wor")
                        for k in range(KT):
                            nc.scalar.dma_start(
                                wor[:, k * 1024:(k + 1) * 1024],
                                wot_d.ap()[k * 128:(k + 1) * 128, 0:1024])
                    outsb = [top_.tile([128, D], F32, name=f"outsb_{t2}")
                             for t2 in range(2)]
                    for nbh in range(2):
                        po = [pst.tile([128, 512], F32, name="po")
                              for _ in range(4)]
                        for ki, k in enumerate(MORDER):
                            if nbh == 0:
                                wo_k = wor[:, k * 1024:(k + 1) * 1024]
                            else:
                                wo_t = wst.tile([128, 1024], BF16, name="wo_k")
                                nc.scalar.dma_start(
                                    wo_t[:],
                                    wot_d.ap()[k * 128:(k + 1) * 128,
                                               1024:2048])
                                wo_k = wo_t[:]
                            st, sp = (ki == 0), (ki == KT - 1)
                            for t2 in range(2):
                                lhsT = oqb[k][:, (ch * 2 + t2) * 128:
                                              (ch * 2 + t2 + 1) * 128]
                                for n2 in range(2):
                                    nc.tensor.matmul(
                                        po[t2 * 2 + n2][:], lhsT,
                                        wo_k[:, n2 * 512:(n2 + 1) * 512],
                                        start=st, stop=sp)
                        for t2 in range(2):
                            tti = ch * 2 + t2
                            for n2 in range(2):
                                nc.scalar.mul(
                                    outsb[t2][:, nbh * 1024 + n2 * 512:
                                              nbh * 1024 + (n2 + 1) * 512],
                                    po[t2 * 2 + n2][:],
                                    coall[:, tti:tti + 1])
                    for t2 in range(2):
                        tti = ch * 2 + t2
                        nc.sync.dma_start(
                            out_d.ap()[tti * 128:(tti + 1) * 128, :],
                            outsb[t2][:])

    nc.compile()
    return nc


_NC_CACHE = None
LAST_RESULTS = None


def _get_nc():
    global _NC_CACHE
    if _NC_CACHE is None:
        _NC_CACHE = build_nc()
    return _NC_CACHE


def _quant_weight(w):
    """fla BitLinear ternary weight quant. w [out, in] f32.
    Returns integer-valued f32 WT [in, out] and the reciprocal scale 1/ws."""
    import jax
    import jax.numpy as jnp

    mean_abs = np.asarray(
        jax.jit(lambda a: jnp.mean(jnp.abs(a)), backend="cpu")(w)
    )
    ws = np.float32(1.0) / np.maximum(mean_abs.astype(np.float32),
                                      np.float32(1e-5))
    wq = np.clip(np.round(w * ws), -1.0, 1.0).astype(np.float32)
    return wq.T.copy(), np.float32(1.0) / ws


def kernel(hidden_states, Wi, Wf, Wg, Wo, g_norm_weight):
    nc = _get_nc()

    wiq, rwsi = _quant_weight(np.asarray(Wi))
    wfq, rwsf = _quant_weight(np.asarray(Wf))
    wgq, rwsg = _quant_weight(np.asarray(Wg))
    woq, rwso = _quant_weight(np.asarray(Wo))

    # lhsT tiles pre-tiled per m: [m][p][k][c] = WT[k*128+p, m*128+c]
    def tile_lhs(wt):
        return np.ascontiguousarray(
            wt.reshape(KT, 128, MT, 128).transpose(2, 1, 0, 3)
        ).astype(ml_dtypes.bfloat16)

    wit = tile_lhs(wiq)
    wft = tile_lhs(wfq)
    wgt = tile_lhs(wgq)
    wot = woq.astype(ml_dtypes.bfloat16)

    gw = np.asarray(g_norm_weight, dtype=np.float32)
    gwc = np.ascontiguousarray(gw.reshape(MT, 128).T)
    x = np.asarray(hidden_states, dtype=np.float32)

    in_maps = []
    for c in range(NCORES):
        b, half = c // 2, c % 2
        rw = np.zeros((128, 6), np.float32)
        rw[:, 0] = rwsi
        rw[:, 1] = rwsf
        rw[:, 2] = -rwsf
        rw[:, 3] = rwsg
        rw[:, 4] = rwso
        rw[:, 5] = rwso / np.float32(127.0)
        in_maps.append({
            "x": np.ascontiguousarray(x[b, half * TPC:(half + 1) * TPC, :]),
            "wit": wit, "wft": wft, "wgt": wgt, "wot": wot,
            "gwc": gwc,
            "mask_even": np.full((128, 1), 1.0 - half, np.float32),
            "mask_odd": np.full((128, 1), float(half), np.float32),
            "rws": rw,
        })

    import os
    trace = bool(os.environ.get("HGRN_TRACE"))
    res = run_bass_kernel_spmd(nc, in_maps, list(range(NCORES)), trace=trace)
    global LAST_RESULTS
    LAST_RESULTS = res
    out = np.empty((B, L, D), np.float32)
    for c in range(NCORES):
        b, half = c // 2, c % 2
        out[b, half * TPC:(half + 1) * TPC, :] = res.results[c]["out"]
    return out


# revision 23
# speedup vs baseline: 1.4528x; 1.0440x over previous
"""HGRN BitAttention Trainium2 kernel (8-core SPMD, token-sharded), v2.

Sharding: core c handles batch c//2, sequence half c%2 (1024 tokens).
The HGRN recurrence carry h[1023] crosses the half boundary via two small
pair-AllReduces (m 0-7 and m 8-15) so most of the collective latency hides
under phase-P compute; masks keep the program uniform (SPMD).

BitLinear trick: activations quantize to integers in [-127,127] and weights
to {-1,0,1} - both exact in bf16 - so all four projections run as
exact-integer bf16 matmuls with fp32 PSUM accumulation.

v2 layout strategy: i/f/g projections all run feature-major ([d_out, tok]
PSUM) off one shared transposed-activation operand; the gate
(FusedRMSNormSwishGate) also runs feature-major, using PE ones-matmuls for
the over-feature sums and a gpsimd partition_all_reduce for the over-feature
abs-max, producing quantized o directly in the [d, tok] layout the output
projection consumes as its stationary operand - no transposes. The gate is
token-chunked (4 x 256) so output-projection matmuls pipeline behind it.
"""

import numpy as np
import ml_dtypes

import concourse.bass as bass
import concourse.bacc as bacc
import concourse.mybir as mybir
import concourse.bass_isa as bass_isa
import concourse.tile as tile
from concourse.bass_utils import run_bass_kernel_spmd

F32 = mybir.dt.float32
BF16 = mybir.dt.bfloat16
F16 = mybir.dt.float16
I32 = mybir.dt.int32
AF = mybir.ActivationFunctionType
OP = mybir.AluOpType

B, L, D = 4, 2048, 2048
NCORES = 8
TPC = L // 2          # tokens per core = 1024
NTT = TPC // 128      # 8 token tiles per core
KT = D // 128         # 16 k tiles
MT = D // 128         # 16 m tiles
NCH = 4               # gate/out token chunks of 256
TCH = TPC // NCH      # 256
EPS = 1e-5


def build_nc():
    nc = bacc.Bacc("TRN2", target_bir_lowering=False, debug=False,
                   num_devices=NCORES)

    x_d = nc.dram_tensor("x", [TPC, D], F32, kind="ExternalInput")
    wit_d = nc.dram_tensor("wit", [MT, 128, KT, 128], BF16, kind="ExternalInput")
    wft_d = nc.dram_tensor("wft", [MT, 128, KT, 128], BF16, kind="ExternalInput")
    wgt_d = nc.dram_tensor("wgt", [MT, 128, KT, 128], BF16, kind="ExternalInput")
    wot_d = nc.dram_tensor("wot", [D, D], BF16, kind="ExternalInput")
    gwc_d = nc.dram_tensor("gwc", [128, MT], F32, kind="ExternalInput")
    me_d = nc.dram_tensor("mask_even", [128, 1], F32, kind="ExternalInput")
    mo_d = nc.dram_tensor("mask_odd", [128, 1], F32, kind="ExternalInput")
    rws_d = nc.dram_tensor("rws", [128, 6], F32, kind="ExternalInput")
    out_d = nc.dram_tensor("out", [TPC, D], F32, kind="ExternalOutput")

    with tile.TileContext(nc) as tc:
        with (
            tc.tile_pool(name="const", bufs=1) as cp,
            tc.tile_pool(name="hp", bufs=1) as hp,
            tc.tile_pool(name="dram", bufs=1, space="DRAM") as dram,
        ):
            # ---- constants ----
            me = cp.tile([128, 1], F32)
            nc.sync.dma_start(me[:], me_d.ap())
            mo = cp.tile([128, 1], F32)
            nc.sync.dma_start(mo[:], mo_d.ap())
            gwc = cp.tile([128, MT], F32)
            nc.sync.dma_start(gwc[:], gwc_d.ap())
            rws = cp.tile([128, 6], F32)
            nc.sync.dma_start(rws[:], rws_d.ap())
            rwsi, rwsf, rwsfn, rwsg, rwso, rwso127 = (
                rws[:, i:i + 1] for i in range(6))
            epsb = cp.tile([128, 1], F32)
            nc.vector.memset(epsb[:], EPS)
            zeros = cp.tile([128, 512], F16)
            nc.vector.memset(zeros[:], 0.0)
            ones1 = cp.tile([1, 128], F32)
            nc.vector.memset(ones1[:], 1.0)
            onesc = cp.tile([128, 1], BF16)
            nc.vector.memset(onesc[:], 1.0)

            srec = cp.tile([128, NTT], F32)     # (1/s_x) per token tile col
            S = cp.tile([128, TPC], F32)        # (1/s_x) broadcast, feat-major
            bnd = cp.tile([128, MT], F32)
            hbnd0 = cp.tile([128, MT], F32)     # f32 h boundary of half 0
            carried = cp.tile([128, MT], F32)
            row4k = cp.tile([1, TPC], F32)      # X: 1/s row; G: rstd_g row
            gsqacc = cp.tile([1, TPC], F32)     # sum over features of g^2
            coall = cp.tile([128, NTT], F32)    # per-token out scale, [p,1] x8

            hs = [None] * MT
            fcs = [None] * MT
            g16 = [None] * MT

            fc_ctx = tc.tile_pool(name="fcp", bufs=1)
            fcp = fc_ctx.__enter__()

            xq_ctx = tc.tile_pool(name="xq", bufs=1)
            xqp = xq_ctx.__enter__()
            # two halves of quantized-x, feature-major: [p, k, t(512)]
            xqh = [xqp.tile([128, KT * 512], BF16, name=f"xqh{h}")
                   for h in range(2)]
            xqh3 = [t[:].rearrange("p (k t) -> p k t", k=KT) for t in xqh]

            wf_ctx = tc.tile_pool(name="wfi", bufs=2)
            wfi = wf_ctx.__enter__()

            def load_w(m):
                wi_sb = wfi.tile([128, KT * 128], BF16, name="wi")
                nc.scalar.dma_start(
                    wi_sb[:], wit_d.ap()[m].rearrange("p k c -> p (k c)"))
                wf_sb = wfi.tile([128, KT * 128], BF16, name="wf")
                nc.scalar.dma_start(
                    wf_sb[:], wft_d.ap()[m].rearrange("p k c -> p (k c)"))
                wg_sb = wfi.tile([128, KT * 128], BF16, name="wg")
                nc.scalar.dma_start(
                    wg_sb[:], wgt_d.ap()[m].rearrange("p k c -> p (k c)"))
                return wi_sb, wf_sb, wg_sb

            # prefetch the first two feature tiles' weights before phase X
            # so projection matmuls can begin as soon as xqh0 lands
            wpre = [load_w(0), load_w(1)]

            # ================= Phase X: normalize + quantize x ==============
            with (
                tc.tile_pool(name="xin", bufs=2) as xin,
                tc.tile_pool(name="xwa", bufs=2) as xwa,
                tc.tile_pool(name="xwb", bufs=1) as xwb,
                tc.tile_pool(name="psx", bufs=2, space="PSUM") as psx,
            ):
                for tt in range(NTT):
                    xt = xin.tile([128, D], F32)
                    nc.sync.dma_start(xt[:], x_d.ap()[tt * 128:(tt + 1) * 128, :])
                    scr = xwb.tile([128, D], F32, name="scr")
                    ssum = xwa.tile([128, 1], F32)
                    nc.scalar.activation(scr[:], xt[:], AF.Square,
                                         accum_out=ssum[:])
                    std = xwa.tile([128, 1], F32)
                    nc.scalar.activation(std[:], ssum[:], AF.Sqrt,
                                         bias=epsb[:], scale=1.0 / D)
                    rstd = xwa.tile([128, 1], F32)
                    nc.vector.reciprocal(rstd[:], std[:])
                    # normalize x in place
                    nc.scalar.mul(xt[:], xt[:], rstd[:])
                    mxn = xwa.tile([128, 1], F32)
                    nc.vector.tensor_reduce(mxn[:], xt[:], mybir.AxisListType.X,
                                            OP.max, apply_absolute_value=True)
                    nc.vector.tensor_scalar_max(mxn[:], mxn[:], EPS)
                    nc.vector.tensor_scalar_mul(srec[:, tt:tt + 1], mxn[:],
                                                1.0 / 127.0)
                    sst = xwa.tile([128, 1], F32)
                    nc.vector.reciprocal(sst[:], mxn[:])
                    nc.vector.tensor_scalar_mul(sst[:], sst[:], 127.0)
                    qi = xwb.tile([128, D], I32, name="qi")
                    nc.scalar.activation(qi[:], xt[:], AF.Identity, scale=sst[:])
                    qb = xwb.tile([128, D], BF16, name="qb")
                    nc.vector.tensor_copy(qb[:], qi[:])
                    h, sub = tt // 4, tt % 4
                    nc.sync.dma_start_transpose(
                        xqh3[h][:, :, sub * 128:(sub + 1) * 128], qb[:])

                # S = broadcast of (1/s) to [128, TPC] feature-major
                srd = dram.tile([1, TPC], F32)
                nc.sync.dma_start(
                    srd[:].rearrange("o (t p) -> (o p) t", p=128), srec[:])
                nc.sync.dma_start(row4k[:], srd[:])
                for c in range(2):
                    pS = psx.tile([128, 512], F32)
                    nc.tensor.matmul(pS[:], ones1[:],
                                     row4k[:, c * 512:(c + 1) * 512],
                                     start=True, stop=True)
                    nc.scalar.copy(S[:, c * 512:(c + 1) * 512], pS[:])

            # ====== Phase P: i/f/g projections + scans (feature-major) ======
            with (
                tc.tile_pool(name="pw", bufs=2) as pw,
                tc.tile_pool(name="pw1", bufs=1) as pw1,
                tc.tile_pool(name="psp", bufs=2, space="PSUM") as psp,
                tc.tile_pool(name="psq", bufs=1, space="PSUM") as psq,
            ):
                for half in range(2):
                    Sh = S[:, half * 512:(half + 1) * 512]
                    gsq_ps = psq.tile([1, 512], F32, name="gsq")
                    # half 1 runs m 8-15 first so its carry exchange issues
                    # early and m 0-7's exchange is the only one at phase end
                    morder = (list(range(MT)) if half == 0 else
                              list(range(MT // 2, MT)) + list(range(MT // 2)))
                    for mi, m in enumerate(morder):
                        if half == 0 and mi < 2:
                            wi_sb, wf_sb, wg_sb = wpre[mi]
                        else:
                            wi_sb, wf_sb, wg_sb = load_w(m)
                        psi = psp.tile([128, 512], F32, name="psi")
                        psf = psp.tile([128, 512], F32, name="psf")
                        psg = psp.tile([128, 512], F32, name="psg")
                        for k in range(KT):
                            rhs = xqh[half][:, k * 512:(k + 1) * 512]
                            st, sp = (k == 0), (k == KT - 1)
                            nc.tensor.matmul(psi[:], wi_sb[:, k * 128:(k + 1) * 128],
                                             rhs, start=st, stop=sp)
                            nc.tensor.matmul(psf[:], wf_sb[:, k * 128:(k + 1) * 128],
                                             rhs, start=st, stop=sp)
                            nc.tensor.matmul(psg[:], wg_sb[:, k * 128:(k + 1) * 128],
                                             rhs, start=st, stop=sp)
                        # f path
                        tmpf = pw.tile([128, 512], F32)
                        nc.vector.tensor_tensor(tmpf[:], psf[:], Sh, OP.mult)
                        G = pw.tile([128, 512], F32)
                        nc.scalar.activation(G[:], tmpf[:], AF.Sigmoid,
                                             scale=rwsfn)
                        F = pw.tile([128, 512], F32)
                        nc.scalar.activation(F[:], tmpf[:], AF.Sigmoid,
                                             scale=rwsf)
                        # i path
                        tmpi = pw.tile([128, 512], F32)
                        nc.vector.tensor_tensor(tmpi[:], psi[:], Sh, OP.mult)
                        sgi = pw1.tile([128, 512], F32)
                        nc.scalar.activation(sgi[:], tmpi[:], AF.Sigmoid,
                                             scale=rwsi)
                        SiL2 = pw.tile([128, 512], F32)
                        nc.vector.scalar_tensor_tensor(SiL2[:], tmpi[:], rwsi,
                                                       sgi[:], OP.mult, OP.mult)
                        Iin = pw.tile([128, 512], F32)
                        nc.vector.tensor_tensor(Iin[:], SiL2[:], G[:], OP.mult)
                        # g path (store dequantized g as f16)
                        if half == 0:
                            g16[m] = hp.tile([128, TPC], F16, name=f"g_{m}")
                            hs[m] = hp.tile([128, TPC], BF16, name=f"h_{m}")
                            fcs[m] = fcp.tile([128, TPC], F16, name=f"fc_{m}")
                        gsl = g16[m][:, half * 512:(half + 1) * 512]
                        nc.vector.scalar_tensor_tensor(gsl, psg[:], rwsg, Sh,
                                                       OP.mult, OP.mult)
                        g2 = pw.tile([128, 512], BF16)
                        nc.scalar.activation(g2[:], gsl, AF.Square)
                        nc.tensor.matmul(gsq_ps[:], onesc[:], g2[:],
                                         start=(mi == 0), stop=(mi == MT - 1))
                        # scans (f32 transient keeps an exact carry boundary)
                        htmp = pw1.tile([128, 512], F32)
                        if half == 0:
                            nc.vector.tensor_tensor_scan(
                                htmp[:], F[:], Iin[:], 0.0, OP.mult, OP.add)
                            nc.vector.tensor_copy(hbnd0[:, m:m + 1],
                                                  htmp[:, 511:512])
                            nc.vector.tensor_tensor_scan(
                                fcs[m][:, 0:512], F[:], zeros[:], 1.0,
                                OP.mult, OP.add)
                        else:
                            nc.vector.tensor_tensor_scan(
                                htmp[:], F[:], Iin[:], hbnd0[:, m:m + 1],
                                OP.mult, OP.add)
                            nc.vector.tensor_copy(bnd[:, m:m + 1],
                                                  htmp[:, 511:512])
                            nc.vector.tensor_tensor_scan(
                                fcs[m][:, 512:1024], F[:], zeros[:],
                                fcs[m][:, 511:512], OP.mult, OP.add)
                        nc.vector.tensor_copy(
                            hs[m][:, half * 512:(half + 1) * 512], htmp[:])

                        # carry exchange, split in two to hide latency
                        if half == 1 and mi in (MT // 2 - 1, MT - 1):
                            lo = MT // 2 if mi < MT // 2 else 0
                            hi = lo + MT // 2
                            bnd2 = cp.tile([128, MT // 2], F32,
                                           name=f"bnd2_{lo}")
                            nc.vector.tensor_scalar_mul(
                                bnd2[:], bnd[:, lo:hi], me[:])
                            cin = dram.tile([128, MT // 2], F32)
                            cout = dram.tile([128, MT // 2], F32)
                            nc.sync.dma_start(cin[:], bnd2[:])
                            nc.gpsimd.collective_compute(
                                "AllReduce", OP.add,
                                replica_groups=[[0, 1], [2, 3], [4, 5], [6, 7]],
                                ins=[cin.opt()], outs=[cout.opt()],
                            )
                            csb = cp.tile([128, MT // 2], F32,
                                          name=f"csb_{lo}")
                            nc.sync.dma_start(csb[:], cout[:])
                            nc.vector.tensor_scalar_mul(
                                carried[:, lo:hi], csb[:], mo[:])
                    # stash this half's g^2 column sums before psum reuse
                    nc.scalar.copy(gsqacc[:, half * 512:(half + 1) * 512],
                                   gsq_ps[:])

            wf_ctx.__exit__(None, None, None)
            xq_ctx.__exit__(None, None, None)

            # carry fixup in place (bf16), then retire the cumprod tiles
            MORDER = list(range(MT // 2, MT)) + list(range(MT // 2))
            for m in MORDER:
                nc.vector.scalar_tensor_tensor(
                    hs[m][:], fcs[m][:], carried[:, m:m + 1], hs[m][:],
                    OP.mult, OP.add)
            fc_ctx.__exit__(None, None, None)

            # ============ Phase G+T: gate + output projection ==============
            with (
                tc.tile_pool(name="gw2", bufs=2) as gp,
                tc.tile_pool(name="gw1", bufs=1) as gp1,
                tc.tile_pool(name="tpp", bufs=1) as tpp,
                tc.tile_pool(name="oqt", bufs=1) as oqtp,
                tc.tile_pool(name="wst", bufs=3) as wst,
                tc.tile_pool(name="tout", bufs=1) as top_,
                tc.tile_pool(name="psg2", bufs=1, space="PSUM") as psg2,
                tc.tile_pool(name="psb", bufs=1, space="PSUM") as psb,
                tc.tile_pool(name="pst", bufs=5, space="PSUM") as pst,
            ):
                # rstd_g row (reuses the X-phase row slot) + broadcast
                nc.scalar.activation(row4k[:], gsqacc[:], AF.Sqrt,
                                     bias=epsb[0:1, :], scale=1.0 / D)
                nc.vector.reciprocal(row4k[:], row4k[:])
                rstdg_b = gp1.tile([128, TPC], F32, name="rstdg_b")
                for c in range(2):
                    pS = psb.tile([128, 512], F32, name="bc")
                    nc.tensor.matmul(pS[:], ones1[:],
                                     row4k[:, c * 512:(c + 1) * 512],
                                     start=True, stop=True)
                    nc.scalar.copy(rstdg_b[:, c * 512:(c + 1) * 512], pS[:])

                oqb = [oqtp.tile([128, TPC], BF16, name=f"oq_{m}")
                       for m in range(MT)]

                for ch in range(NCH):
                    tsl = slice(ch * TCH, (ch + 1) * TCH)
                    # --- gate sweep over feature tiles ---
                    tmps = {}
                    macc = gp1.tile([128, TCH], F32, name="macc")
                    osq_ps = psg2.tile([1, TCH], F32, name="osq")
                    for mi, m in enumerate(MORDER):
                        hsw = gp.tile([128, TCH], F32, name="hsw")
                        nc.scalar.activation(hsw[:], hs[m][:, tsl], AF.Silu)
                        tmp = tpp.tile([128, TCH], F32, name=f"tmp_{m}")
                        nc.vector.scalar_tensor_tensor(
                            tmp[:], g16[m][:, tsl], gwc[:, m:m + 1], hsw[:],
                            OP.mult, OP.mult)
                        tmps[m] = tmp
                        tmp2 = gp.tile([128, TCH], F32, name="tmp2")
                        nc.scalar.activation(tmp2[:], tmp[:], AF.Square)
                        if mi == 0:
                            nc.vector.tensor_copy(macc[:], tmp2[:])
                        else:
                            nc.vector.tensor_tensor(macc[:], macc[:], tmp2[:],
                                                    OP.max)
                        tmp2b = gp.tile([128, TCH], BF16, name="tmp2b")
                        nc.vector.tensor_copy(tmp2b[:], tmp2[:])
                        nc.tensor.matmul(osq_ps[:], onesc[:], tmp2b[:],
                                         start=(mi == 0), stop=(mi == MT - 1))
                    # --- per-token scale rows ---
                    pmx = gp1.tile([128, TCH], F32, name="pmx")
                    nc.gpsimd.partition_all_reduce(pmx[:], macc[:], 128,
                                                   bass_isa.ReduceOp.max)
                    mxt = gp1.tile([1, TCH], F32, name="mxt")
                    nc.scalar.activation(mxt[:], pmx[0:1, :], AF.Sqrt)
                    mo2 = gp1.tile([1, TCH], F32, name="mo2")
                    nc.vector.tensor_tensor(mo2[:], osq_ps[:],
                                            row4k[0:1, tsl], OP.mult)
                    nc.vector.tensor_tensor(mo2[:], mo2[:],
                                            row4k[0:1, tsl], OP.mult)
                    rso = gp1.tile([1, TCH], F32, name="rso")
                    nc.scalar.activation(rso[:], mo2[:], AF.Sqrt,
                                         bias=epsb[0:1, :], scale=1.0 / D)
                    nc.vector.reciprocal(rso[:], rso[:])
                    mxn = gp1.tile([1, TCH], F32, name="mxn")
                    nc.vector.tensor_tensor(mxn[:], mxt[:],
                                            row4k[0:1, tsl], OP.mult)
                    nc.vector.tensor_tensor(mxn[:], mxn[:], rso[:], OP.mult)
                    nc.vector.tensor_scalar_max(mxn[:], mxn[:], EPS)
                    # output scale row -> [p,1] columns via PE transpose
                    osc = gp1.tile([1, TCH], F32, name="osc")
                    nc.vector.tensor_scalar_mul(osc[:], mxn[:],
                                                rwso127[0:1, :])
                    for t2 in range(2):
                        tti = ch * 2 + t2
                        ptc = psb.tile([128, 1], F32, name="ptc")
                        nc.tensor.matmul(ptc[:],
                                         osc[:, t2 * 128:(t2 + 1) * 128],
                                         ones1[:, 0:1], is_transpose=True,
                                         start=True, stop=True)
                        nc.scalar.copy(coall[:, tti:tti + 1], ptc[:])
                    # qmul = (127 / mxn) * rstd_o * rstd_g
                    qm = gp1.tile([1, TCH], F32, name="qm")
                    nc.vector.reciprocal(qm[:], mxn[:])
                    nc.vector.tensor_scalar_mul(qm[:], qm[:], 127.0)
                    nc.vector.tensor_tensor(qm[:], qm[:], rso[:], OP.mult)
                    nc.vector.tensor_tensor(qm[:], qm[:], row4k[0:1, tsl],
                                            OP.mult)
                    qmb = gp.tile([128, TCH], F32, name="qmb")
                    pq = psb.tile([128, TCH], F32, name="bc")
                    nc.tensor.matmul(pq[:], ones1[:], qm[:],
                                     start=True, stop=True)
                    nc.scalar.copy(qmb[:], pq[:])
                    # --- quantize o (feature-major: already the lhsT layout) ---
                    for m in MORDER:
                        oqi = gp.tile([128, TCH], I32, name="oqi")
                        nc.vector.tensor_tensor(oqi[:], tmps[m][:], qmb[:],
                                                OP.mult)
                        nc.scalar.copy(oqb[m][:, tsl], oqi[:])
                    # --- output projection for this chunk's 2 token tiles ---
                    if ch == 0:
                        # cache the low half of Wo in SBUF for all chunks
                        wor = gp1.tile([128, KT * 1024], BF16, name="wor")
                        for k in range(KT):
                            nc.sync.dma_start(
                                wor[:, k * 1024:(k + 1) * 1024],
                                wot_d.ap()[k * 128:(k + 1) * 128, 0:1024])
                    outsb = [top_.tile([128, D], F32, name=f"outsb_{t2}")
                             for t2 in range(2)]
                    for nbh in range(2):
                        po = [pst.tile([128, 512], F32, name="po")
                              for _ in range(4)]
                        for ki, k in enumerate(MORDER):
                            if nbh == 0:
                                wo_k = wor[:, k * 1024:(k + 1) * 1024]
                            else:
                                wo_t = wst.tile([128, 1024], BF16, name="wo_k")
                                nc.sync.dma_start(
                                    wo_t[:],
                                    wot_d.ap()[k * 128:(k + 1) * 128,
                                               1024:2048])
                                wo_k = wo_t[:]
                            st, sp = (ki == 0), (ki == KT - 1)
                            for t2 in range(2):
                                lhsT = oqb[k][:, (ch * 2 + t2) * 128:
                                              (ch * 2 + t2 + 1) * 128]
                                for n2 in range(2):
                                    nc.tensor.matmul(
                                        po[t2 * 2 + n2][:], lhsT,
                                        wo_k[:, n2 * 512:(n2 + 1) * 512],
                                        start=st, stop=sp)
                        for t2 in range(2):
                            tti = ch * 2 + t2
                            for n2 in range(2):
                                nc.scalar.mul(
                                    outsb[t2][:, nbh * 1024 + n2 * 512:
                                              nbh * 1024 + (n2 + 1) * 512],
                                    po[t2 * 2 + n2][:],
                                    coall[:, tti:tti + 1])
                    for t2 in range(2):
                        tti = ch * 2 + t2
                        nc.sync.dma_start(
                            out_d.ap()[tti * 128:(tti + 1) * 128, :],
                            outsb[t2][:])

    nc.compile()
    return nc


_NC_CACHE = None
LAST_RESULTS = None


def _get_nc():
    global _NC_CACHE
    if _NC_CACHE is None:
        _NC_CACHE = build_nc()
    return _NC_CACHE


def _quant_weight(w):
    """fla BitLinear ternary weight quant. w [out, in] f32.
    Returns integer-valued f32 WT [in, out] and the reciprocal scale 1/ws."""
    import jax
    import jax.numpy as jnp

    mean_abs = np.asarray(
        jax.jit(lambda a: jnp.mean(jnp.abs(a)), backend="cpu")(w)
    )
    ws = np.float32(1.0) / np.maximum(mean_abs.astype(np.float32),
                                      np.float32(1e-5))
    wq = np.clip(np.round(w * ws), -1.0, 1.0).astype(np.float32)
    return wq.T.copy(), np.float32(1.0) / ws


def kernel(hidden_states, Wi, Wf, Wg, Wo, g_norm_weight):
    nc = _get_nc()

    wiq, rwsi = _quant_weight(np.asarray(Wi))
    wfq, rwsf = _quant_weight(np.asarray(Wf))
    wgq, rwsg = _quant_weight(np.asarray(Wg))
    woq, rwso = _quant_weight(np.asarray(Wo))

    # lhsT tiles pre-tiled per m: [m][p][k][c] = WT[k*128+p, m*128+c]
    def tile_lhs(wt):
        return np.ascontiguousarray(
            wt.reshape(KT, 128, MT, 128).transpose(2, 1, 0, 3)
        ).astype(ml_dtypes.bfloat16)

    wit = tile_lhs(wiq)
    wft = tile_lhs(wfq)
    wgt = tile_lhs(wgq)
    wot = woq.astype(ml_dtypes.bfloat16)

    gw = np.asarray(g_norm_weight, dtype=np.float32)
    gwc = np.ascontiguousarray(gw.reshape(MT, 128).T)
    x = np.asarray(hidden_states, dtype=np.float32)

    in_maps = []
    for c in range(NCORES):
        b, half = c // 2, c % 2
        rw = np.zeros((128, 6), np.float32)
        rw[:, 0] = rwsi
        rw[:, 1] = rwsf
        rw[:, 2] = -rwsf
        rw[:, 3] = rwsg
        rw[:, 4] = rwso
        rw[:, 5] = rwso / np.float32(127.0)
        in_maps.append({
            "x": np.ascontiguousarray(x[b, half * TPC:(half + 1) * TPC, :]),
            "wit": wit, "wft": wft, "wgt": wgt, "wot": wot,
            "gwc": gwc,
            "mask_even": np.full((128, 1), 1.0 - half, np.float32),
            "mask_odd": np.full((128, 1), float(half), np.float32),
            "rws": rw,
        })

    import os
    trace = bool(os.environ.get("HGRN_TRACE"))
    res = run_bass_kernel_spmd(nc, in_maps, list(range(NCORES)), trace=trace)
    global LAST_RESULTS
    LAST_RESULTS = res
    out = np.empty((B, L, D), np.float32)
    for c in range(NCORES):
        b, half = c // 2, c % 2
        out[b, half * TPC:(half + 1) * TPC, :] = res.results[c]["out"]
    return out
